# revision 1
# baseline (speedup 1.0000x reference)
"""Trainium2 Bass kernel for nn_MIPS_74904229642848 (v2).

Pipeline (8 NeuronCores, SPMD, batch-sharded 2 rows/core, S=4 streams/core):
  1. 2-layer bidirectional LSTM, all-bf16 matmuls. Per step: one identity
     matmul injects the precomputed input-gate terms into PSUM (chunked so
     each recurrence starts after its first xg chunk), four bf16 block-diag
     recurrence matmuls accumulate on top; sigmoid/tanh on ACT, cell math
     on DVE with c and tanh(c) resident in PSUM.
  2. L2 normalization via ln/exp rsqrt (no Newton), fused scale+pack.
  3. Windowed index sampling of z2 via indirect DMA (bf16).
  4. AllGather of the B-side embeddings only (bf16).
  5. One-pass logits sweep: bf16 sim matmul blocks, exp (ACT, accum_out
     gives row sums), ones-matmul accumulates column sums in PSUM across
     row blocks. Row-lse finished on device; per-core column-sum partials
     shipped to the host, which does the final ln+sum combine.
"""

import numpy as np

_D, _E, _H, _B, _W = 64, 128, 64, 16, 3
_T = 512
_TEMP = 0.05
_NCORES = 8
_BS = _B // _NCORES          # batch rows per core
_S = 2 * _BS                 # streams per core: (x1,b0),(x1,b1),(x2,b0),(x2,b1)
_GF = 4 * _S                 # gate-block width per step

# torch gate order i,f,g,o -> kernel order i,f,o,g (tanh block last)
_GPERM = [0, 1, 3, 2]
_GTANH = 3                   # index of the g gate in kernel order

_cache = {}


def _build(T, dbg=False):
    import concourse.bass as bass
    import concourse.mybir as mybir
    import concourse.tile as tile
    from concourse import bacc
    from concourse.masks import make_identity

    f32 = mybir.dt.float32
    bf16 = mybir.dt.bfloat16
    i32 = mybir.dt.int32
    AF = mybir.ActivationFunctionType
    OP = mybir.AluOpType

    S = _S
    GF = _GF
    TS = T * S
    NLOC = _BS * T
    NGLOB = _NCORES * NLOC
    FC = min(512, TS)            # xg free chunk
    NFC = TS // FC
    TCH = FC // S                # timesteps per xg chunk
    CC = min(512, NGLOB)         # logits col chunk
    NCC = NGLOB // CC
    NRC = (NLOC + 127) // 128    # logits row chunks (M=128)
    NTC = T // 128               # transpose chunks per stream
    NUC = T // 128

    nc = bacc.Bacc("TRN2", target_bir_lowering=False, debug=False,
                   num_devices=_NCORES)

    # ---- I/O ----
    x_in = nc.dram_tensor("x_cat", [_D, TS], bf16, kind="ExternalInput")
    xr_in = nc.dram_tensor("x_rev", [_D, TS], bf16, kind="ExternalInput")
    u_in = nc.dram_tensor("u2", [_BS, T], f32, kind="ExternalInput")
    wi0_in = nc.dram_tensor("wi0T", [_D, 512], bf16, kind="ExternalInput")
    wi1_in = nc.dram_tensor("wi1T", [_E, 512], bf16, kind="ExternalInput")
    wh0_in = nc.dram_tensor("wh0bd", [_E, 512], bf16, kind="ExternalInput")
    wh1_in = nc.dram_tensor("wh1bd", [_E, 512], bf16, kind="ExternalInput")
    b0_in = nc.dram_tensor("bias0", [_E, 4], f32, kind="ExternalInput")
    b1_in = nc.dram_tensor("bias1", [_E, 4], f32, kind="ExternalInput")
    out_d = nc.dram_tensor("outp", [128, 4], f32, kind="ExternalOutput")
    cs_d = nc.dram_tensor("colsum", [1, NGLOB], f32, kind="ExternalOutput")
    if dbg:
        anb_d = nc.dram_tensor("anb", [128, 4 * NLOC], f32,
                               kind="ExternalOutput")
        idx_d = nc.dram_tensor("idxo", [128, _BS * NUC], i32,
                               kind="ExternalOutput")

    # DRAM scratch (offset-0 tensors required for indirect DMA sources)
    z2T_d = [nc.dram_tensor(f"z2T_{b}", [T, _E], bf16) for b in range(_BS)]
    ag_in_d = nc.dram_tensor("ag_in", [128, NLOC], bf16)
    ag_out_d = nc.dram_tensor("ag_out", [_NCORES * 128, NLOC], bf16,
                              addr_space="Shared")

    with tile.TileContext(nc) as tc:
        with (
            tc.tile_pool(name="consts", bufs=1) as consts,
            tc.tile_pool(name="bigbuf", bufs=1) as bigbuf,
            tc.tile_pool(name="state", bufs=1) as state,
            tc.tile_pool(name="small", bufs=3) as small,
            tc.tile_pool(name="nrm", bufs=1) as nrm,
            tc.tile_pool(name="sg", bufs=4) as sgp,
        ):
            dma = nc.sync.dma_start

            # ---------- load constants / inputs ----------
            x_sb = bigbuf.tile([_D, TS], bf16, tag="x")
            xr_sb = bigbuf.tile([_D, TS], bf16, tag="xr")
            dma(out=x_sb[:], in_=x_in.ap())
            dma(out=xr_sb[:], in_=xr_in.ap())
            wi0_sb = consts.tile([_D, 512], bf16)
            wi1_sb = consts.tile([_E, 512], bf16)
            wh0_sb = consts.tile([_E, 512], bf16)
            wh1_sb = consts.tile([_E, 512], bf16)
            b0_sb = consts.tile([_E, 4], f32)
            b1_sb = consts.tile([_E, 4], f32)
            for sb, di in ((wi0_sb, wi0_in), (wi1_sb, wi1_in),
                           (wh0_sb, wh0_in), (wh1_sb, wh1_in),
                           (b0_sb, b0_in), (b1_sb, b1_in)):
                dma(out=sb[:], in_=di.ap())
            ident = consts.tile([128, 128], bf16)
            make_identity(nc, ident[:])
            ones_col = consts.tile([128, 1], bf16)
            nc.vector.memset(ones_col[:], 1.0)
            ones_row = consts.tile([1, 128], f32)
            nc.vector.memset(ones_row[:], 1.0)

            # u in [128, (b, c)] layout: t = c*128 + p
            u8 = consts.tile([128, _BS * NUC], f32)
            uap = u_in.ap()
            u_src = bass.AP(tensor=uap.tensor, offset=uap.offset,
                            ap=[[1, 128], [T, _BS], [128, NUC]])
            dma(out=u8[:], in_=u_src)

            # ---------- window indices on device (only needs u) ----------
            t8i = consts.tile([128, _BS * NUC], i32)
            nc.gpsimd.iota(t8i[:], pattern=[[0, _BS], [128, NUC]], base=0,
                           channel_multiplier=1)
            tf = small.tile([128, _BS * NUC], f32, tag="idxf")
            nc.vector.tensor_copy(tf[:], t8i[:])
            ks = small.tile([128, _BS * NUC], f32, tag="ks")
            nc.vector.tensor_scalar_mul(ks[:], tf[:], 1.0 / (T - 1))
            # the oracle's (k*s).astype(int32) rounds-to-nearest on neuron,
            # so the carry fires at 0.5 rather than 1.0
            nc.vector.tensor_scalar(ks[:], ks[:], 0.5, None, OP.is_ge)
            nc.vector.tensor_add(tf[:], tf[:], ks[:])          # center
            lo = small.tile([128, _BS * NUC], f32, tag="lo")
            nc.vector.tensor_scalar(lo[:], tf[:], -float(_W), 0.0,
                                    OP.add, OP.max)
            hi = small.tile([128, _BS * NUC], f32, tag="hi")
            nc.vector.tensor_scalar(hi[:], tf[:], float(_W), float(T),
                                    OP.add, OP.min)
            cnt = small.tile([128, _BS * NUC], f32, tag="cnt")
            nc.vector.tensor_sub(cnt[:], hi[:], lo[:])
            pr = small.tile([128, _BS * NUC], f32, tag="pr")
            nc.vector.tensor_mul(pr[:], u8[:], cnt[:])
            # floor(pr) for pr in [0, 2W+1): sum of is_ge thresholds
            fr = small.tile([128, _BS * NUC], f32, tag="fr")
            nc.vector.memset(fr[:], 0.0)
            for kth in range(1, 2 * _W + 1):
                nc.vector.scalar_tensor_tensor(
                    fr[:], pr[:], float(kth), fr[:], OP.is_ge, OP.add)
            nc.vector.tensor_add(lo[:], lo[:], fr[:])          # i2 (pre-min)
            nc.vector.tensor_scalar_add(hi[:], hi[:], -1.0)
            nc.vector.tensor_tensor(lo[:], lo[:], hi[:], op=OP.min)
            idx = consts.tile([128, _BS * NUC], i32)
            nc.vector.tensor_copy(idx[:], lo[:])

            # ---------- LSTM ----------
            def xg_precompute(ps_big, wiT_sb, K, srcs_f, srcs_b, bias_sb,
                              XGs):
                # XG layout: [128, (t, gate, s)] with gf = GF per step,
                # chunked into NFC tiles so the recurrence can start as
                # soon as chunk 0 is written
                for ch in range(NFC):
                    XG3 = XGs[ch][:].rearrange("p (t gf) -> p t gf", gf=GF)
                    for g in range(4):
                        ps = ps_big.tile([128, FC], f32, tag="xgps")
                        nc.tensor.matmul(
                            out=ps[0:_H, :],
                            lhsT=wiT_sb[0:K, g * 64:(g + 1) * 64],
                            rhs=srcs_f[ch],
                            start=True, stop=True)
                        nc.tensor.matmul(
                            out=ps[_H:128, :],
                            lhsT=wiT_sb[0:K, 256 + g * 64:256 + (g + 1) * 64],
                            rhs=srcs_b[ch],
                            start=True, stop=True)
                        dst = XG3[:, :, g * S:(g + 1) * S]
                        nc.scalar.activation(
                            dst,
                            ps[:].rearrange("p (t s) -> p t s", s=S),
                            AF.Identity, bias=bias_sb[:, g:g + 1])

            def recurrence(ps_gate, ps_state, ps_sg, junkbuf,
                           wh_sb, XGs, Ht, Hr=None):
                Hst = state.tile([128, S], bf16, tag="hst")
                # c and tanh(c) live in PSUM: ACT's PSUM access is faster
                # and the cell math runs on DVE (PSUM-capable)
                Cps = ps_state.tile([128, 2 * S], f32, tag="cps")
                nc.vector.memset(Hst[:], 0.0)
                nc.vector.memset(Cps[:, 0:S], 0.0)
                for t in range(T):
                    gb = ps_gate.tile([128, GF], f32, tag="gates")
                    xgsl = XGs[t // TCH][:, (t % TCH) * GF:
                                         (t % TCH + 1) * GF]
                    # inject xg via identity matmul (clears has_written)
                    nc.tensor.matmul(out=gb[:], lhsT=ident[:],
                                     rhs=xgsl,
                                     start=True, stop=False,
                                     skip_group_check=True)
                    for g in range(4):
                        nc.tensor.matmul(
                            out=gb[:, g * S:(g + 1) * S],
                            lhsT=wh_sb[:, g * 128:(g + 1) * 128],
                            rhs=Hst[:],
                            start=False, stop=(g == 3),
                            skip_group_check=True)
                    # ACT order sigma_if -> tanh_g -> sigma_o: m2 runs in
                    # tanh_g's shadow, m1 fires right after tanh_g, and
                    # sigma_o lands before h needs it. sigma_if and tanh_g
                    # land in two separate PSUM banks (faster ACT access;
                    # separate so ACT never writes a bank DVE is reading);
                    # sigma_o goes to SBUF, off the critical path.
                    sgA = sgp.tile([128, 2 * S], bf16, tag="sgA")
                    sgB = ps_sg.tile([128, S], f32, tag="sgB")
                    sgO = sgp.tile([128, S], bf16, tag="sgO")
                    nc.scalar.activation(sgA[:], gb[:, 0:2 * S], AF.Sigmoid)
                    nc.scalar.activation(sgB[:], gb[:, 3 * S:GF], AF.Tanh)
                    nc.scalar.activation(sgO[:], gb[:, 2 * S:3 * S],
                                         AF.Sigmoid)
                    m2 = sgp.tile([128, S], f32, tag="m2")
                    nc.vector.tensor_mul(m2[:], sgA[:, S:2 * S], Cps[:, 0:S])
                    m1 = sgp.tile([128, S], f32, tag="m1")
                    nc.vector.tensor_mul(m1[:], sgA[:, 0:S], sgB[:])
                    nc.vector.tensor_add(Cps[:, 0:S], m1[:], m2[:])
                    nc.scalar.activation(Cps[:, S:2 * S], Cps[:, 0:S],
                                         AF.Tanh)
                    nc.vector.tensor_mul(Hst[:], sgO[:], Cps[:, S:2 * S])
                    rt = T - 1 - t
                    nc.gpsimd.tensor_copy(Ht[0:_H, t * S:(t + 1) * S],
                                          Hst[0:_H, :])
                    nc.gpsimd.tensor_copy(Ht[_H:128, rt * S:(rt + 1) * S],
                                          Hst[_H:128, :])
                    if Hr is not None:
                        nc.gpsimd.tensor_copy(Hr[0:_H, rt * S:(rt + 1) * S],
                                              Hst[0:_H, :])
                        nc.gpsimd.tensor_copy(Hr[_H:128, t * S:(t + 1) * S],
                                              Hst[_H:128, :])

            H0t = bigbuf.tile([128, TS], bf16, tag="h0t")
            H1t = bigbuf.tile([128, TS], bf16, tag="h1t")
            with (
                tc.tile_pool(name="ps_big", bufs=2, space="PSUM") as ps_big,
                tc.tile_pool(name="ps_gate", bufs=2, space="PSUM") as ps_gate,
                tc.tile_pool(name="ps_state", bufs=1, space="PSUM") as ps_state,
                tc.tile_pool(name="ps_sg", bufs=1, space="PSUM") as ps_sg,
            ):
                XGs = [bigbuf.tile([128, TCH * GF], bf16, tag=f"xg{ch}",
                                   name=f"xg{ch}")
                       for ch in range(NFC)]
                H0r = bigbuf.tile([128, TS], bf16, tag="h0r")
                srcs_f = [x_sb[:, ch * FC:(ch + 1) * FC] for ch in range(NFC)]
                srcs_b = [xr_sb[:, ch * FC:(ch + 1) * FC] for ch in range(NFC)]
                xg_precompute(ps_big, wi0_sb, _D, srcs_f, srcs_b, b0_sb, XGs)
                recurrence(ps_gate, ps_state, ps_sg, None,
                           wh0_sb, XGs, H0t, H0r)

                # layer 2: fwd reads H0t, bwd reads the reversed copy H0r
                # (negative-stride APs are rejected by the BIR verifier)
                srcs_f = [H0t[:, ch * FC:(ch + 1) * FC] for ch in range(NFC)]
                srcs_b = [H0r[:, ch * FC:(ch + 1) * FC] for ch in range(NFC)]
                xg_precompute(ps_big, wi1_sb, _E, srcs_f, srcs_b, b1_sb, XGs)
                recurrence(ps_gate, ps_state, ps_sg, None,
                           wh1_sb, XGs, H1t)

            # ---------- normalize + pack An (z1) / Z2 ----------
            An = bigbuf.tile([128, NLOC], bf16, tag="an")
            Z2 = bigbuf.tile([128, NLOC], bf16, tag="z2")
            H13 = H1t[:].rearrange("p (t s) -> p t s", s=S)
            n2 = nrm.tile([1, TS], f32, tag="n2")
            sq = bigbuf.tile([128, FC], bf16, tag="sq")
            with (
                tc.tile_pool(name="ps_nrm", bufs=2, space="PSUM") as ps_nrm,
                tc.tile_pool(name="ps_row", bufs=2, space="PSUM") as ps_row,
            ):
                for ch in range(NFC):
                    nc.vector.tensor_mul(sq[:],
                                         H1t[:, ch * FC:(ch + 1) * FC],
                                         H1t[:, ch * FC:(ch + 1) * FC])
                    psn = ps_row.tile([1, FC], f32, tag="psn")
                    nc.tensor.matmul(out=psn[:], lhsT=ones_col[:],
                                     rhs=sq[:], start=True, stop=True)
                    nc.vector.tensor_copy(n2[:, ch * FC:(ch + 1) * FC],
                                          psn[:])
                nc.vector.tensor_scalar_max(n2[:], n2[:], 1e-24)
                lnb = nrm.tile([1, TS], f32, tag="lnb")
                nc.scalar.activation(lnb[:], n2[:], AF.Ln)
                rin = nrm.tile([1, TS], f32, tag="rin")
                nc.scalar.activation(rin[:], lnb[:], AF.Exp, scale=-0.5)
                # pack Z2 first so the gather+AllGather chain starts
                # as early as possible; An is packed during the collective
                def pack(streams, ch):
                    psb = ps_nrm.tile([128, FC], f32, tag="nps")
                    nc.tensor.matmul(out=psb[:], lhsT=ones_row[:],
                                     rhs=rin[:, ch * FC:(ch + 1) * FC],
                                     start=True, stop=True)
                    ps3 = psb[:].rearrange("p (t s) -> p t s", s=S)
                    h3 = H13[:, ch * TCH:(ch + 1) * TCH, :]
                    for s in streams:
                        if s < _BS:
                            dst = An[:, s * T + ch * TCH:
                                     s * T + (ch + 1) * TCH]
                        else:
                            b = s - _BS
                            dst = Z2[:, b * T + ch * TCH:
                                     b * T + (ch + 1) * TCH]
                        nc.vector.tensor_tensor(
                            dst,
                            h3[:, :, s:s + 1].rearrange("p t o -> p (t o)"),
                            ps3[:, :, s:s + 1].rearrange("p t o -> p (t o)"),
                            op=OP.mult)
                for ch in range(NFC):
                    pack(range(_BS, S), ch)

            # ---------- gather z2 -> Bn ----------
            Bn = bigbuf.tile([128, NLOC], bf16, tag="bn")
            with tc.tile_pool(name="ps_t", bufs=2, space="PSUM") as ps_t:
                for b in range(_BS):
                    for c in range(NTC):
                        pst = ps_t.tile([128, 128], bf16, tag="tps")
                        nc.tensor.transpose(
                            pst[:],
                            Z2[:, b * T + c * 128:b * T + (c + 1) * 128],
                            ident[:])
                        zt = small.tile([128, 128], bf16, tag="zt")
                        nc.vector.tensor_copy(zt[:], pst[:])
                        dma(out=z2T_d[b].ap()[c * 128:(c + 1) * 128, :],
                            in_=zt[:])
                    for c in range(NTC):
                        gt = small.tile([128, 128], bf16, tag="gt")
                        nc.gpsimd.indirect_dma_start(
                            out=gt[:], out_offset=None,
                            in_=z2T_d[b].ap(),
                            in_offset=bass.IndirectOffsetOnAxis(
                                ap=idx[:, b * NUC + c:b * NUC + c + 1],
                                axis=0))
                        pst = ps_t.tile([128, 128], bf16, tag="tps")
                        nc.tensor.transpose(pst[:], gt[:], ident[:])
                        nc.vector.tensor_copy(
                            Bn[:, b * T + c * 128:b * T + (c + 1) * 128],
                            pst[:])

            # ---------- AllGather (B side only, bf16) ----------
            dma(out=ag_in_d.ap(), in_=Bn[:])
            nc.gpsimd.collective_compute(
                "AllGather", OP.bypass,
                replica_groups=[list(range(_NCORES))],
                ins=[ag_in_d.ap().opt()],
                outs=[ag_out_d.ap().opt()])
            # pack An + compute diag while the collective runs
            with tc.tile_pool(name="ps_nrm2", bufs=2, space="PSUM") as ps_n2:
                def pack2(ch):
                    psb = ps_n2.tile([128, FC], f32, tag="nps2")
                    nc.tensor.matmul(out=psb[:], lhsT=ones_row[:],
                                     rhs=rin[:, ch * FC:(ch + 1) * FC],
                                     start=True, stop=True)
                    ps3 = psb[:].rearrange("p (t s) -> p t s", s=S)
                    h3 = H13[:, ch * TCH:(ch + 1) * TCH, :]
                    for st in range(_BS):
                        dst = An[:, st * T + ch * TCH:
                                 st * T + (ch + 1) * TCH]
                        nc.vector.tensor_tensor(
                            dst,
                            h3[:, :, st:st + 1].rearrange("p t o -> p (t o)"),
                            ps3[:, :, st:st + 1].rearrange(
                                "p t o -> p (t o)"),
                            op=OP.mult)
                for ch in range(NFC):
                    pack2(ch)

            Ball = bigbuf.tile([128, NGLOB], bf16, tag="ball")
            for jj in range(_NCORES):
                dma(out=Ball[:, jj * NLOC:(jj + 1) * NLOC],
                    in_=ag_out_d.ap()[jj * 128:(jj + 1) * 128, :])

            # ---------- output partials ----------
            outp = consts.tile([128, 4], f32)
            nc.vector.memset(outp[:], 0.0)

            with (
                tc.tile_pool(name="ps_d", bufs=1, space="PSUM") as ps_d,
                tc.tile_pool(name="ps_s", bufs=2, space="PSUM") as ps_s,
                tc.tile_pool(name="ps_c", bufs=1, space="PSUM") as ps_c,
            ):
                # diag: sum_i <An_i, Bn_i>
                dg = bigbuf.tile([128, NLOC], bf16, tag="dg")
                nc.vector.tensor_mul(dg[:], An[:], Bn[:])
                ndc = (NLOC + 511) // 512
                psd = ps_d.tile([1, 512], f32, tag="psd")
                for ch in range(ndc):
                    nc.tensor.matmul(out=psd[:], lhsT=ones_col[:],
                                     rhs=dg[:, ch * 512:(ch + 1) * 512],
                                     start=(ch == 0), stop=(ch == ndc - 1))
                nc.vector.reduce_sum(outp[0:1, 2:3], psd[:],
                                     axis=mybir.AxisListType.X)

                # one-pass row+col logsumexp sweep ([128,1024] exp
                # blocks; row sums on DVE, col sums via ones-matmuls)
                CP = 2 * CC
                NCP = NGLOB // CP
                rows = bigbuf.tile([128, NRC * NCP], f32, tag="rows")
                csum = nrm.tile([1, NGLOB], f32, tag="csum")
                for cb in range(NCP):
                    csp0 = ps_c.tile([1, CC], f32, tag="csp0")
                    csp1 = ps_c.tile([1, CC], f32, tag="csp1")
                    for rc in range(NRC):
                        ps = ps_s.tile([128, CP], f32, tag="sps")
                        nc.tensor.matmul(
                            out=ps[:, 0:CC],
                            lhsT=An[:, rc * 128:(rc + 1) * 128],
                            rhs=Ball[:, cb * CP:cb * CP + CC],
                            start=True, stop=True)
                        nc.tensor.matmul(
                            out=ps[:, CC:CP],
                            lhsT=An[:, rc * 128:(rc + 1) * 128],
                            rhs=Ball[:, cb * CP + CC:(cb + 1) * CP],
                            start=True, stop=True)
                        eb = sgp.tile([128, CP], bf16, tag="eb")
                        nc.scalar.activation(
                            eb[:], ps[:], AF.Exp, scale=1.0 / _TEMP)
                        nc.vector.reduce_sum(
                            rows[:, rc * NCP + cb:rc * NCP + cb + 1],
                            eb[:], axis=mybir.AxisListType.X)
                        nc.tensor.matmul(
                            out=csp0[:], lhsT=ones_col[:], rhs=eb[:, 0:CC],
                            start=(rc == 0), stop=(rc == NRC - 1))
                        nc.tensor.matmul(
                            out=csp1[:], lhsT=ones_col[:], rhs=eb[:, CC:CP],
                            start=(rc == 0), stop=(rc == NRC - 1))
                    nc.vector.tensor_copy(csum[:, cb * CP:cb * CP + CC],
                                          csp0[:])
                    nc.vector.tensor_copy(
                        csum[:, cb * CP + CC:(cb + 1) * CP], csp1[:])

                tot = small.tile([128, NRC], f32, tag="tot")
                for rc in range(NRC):
                    nc.vector.reduce_sum(tot[:, rc:rc + 1],
                                         rows[:, rc * NCP:(rc + 1) * NCP],
                                         axis=mybir.AxisListType.X)
                lse = small.tile([128, NRC], f32, tag="lse")
                nc.scalar.activation(lse[:], tot[:], AF.Ln)
                nc.vector.reduce_sum(outp[:, 0:1], lse[:],
                                     axis=mybir.AxisListType.X)

            dma(out=out_d.ap(), in_=outp[:])
            dma(out=cs_d.ap(), in_=csum[:])
            if dbg:
                anb32 = bigbuf.tile([128, NLOC], f32, tag="anb32")
                for i, src in enumerate((An, Bn, Z2)):
                    nc.vector.tensor_copy(anb32[:], src[:])
                    dma(out=anb_d.ap()[:, i * NLOC:(i + 1) * NLOC],
                        in_=anb32[:])
                nc.vector.tensor_copy(anb32[:], H1t[:, 0:NLOC])
                dma(out=anb_d.ap()[:, 3 * NLOC:4 * NLOC], in_=anb32[:])
                dma(out=idx_d.ap(), in_=idx[:])

    nc.compile()
    return nc


def _host_prep(x1, x2, u, wih0, whh0, bih0, bhh0, wih1, whh1, bih1, bhh1, T):
    """Build per-core input maps (all host work is pure data layout)."""
    import ml_dtypes
    bf16 = ml_dtypes.bfloat16

    def gate_stack_T(w):
        # w: [2, 256, K] -> [K, 512]; cols = dir*256 + gperm_gate*64 + j
        K = w.shape[2]
        out = np.empty((K, 512), np.float32)
        for d in range(2):
            for gi, g in enumerate(_GPERM):
                out[:, d * 256 + gi * 64:d * 256 + (gi + 1) * 64] = \
                    w[d, g * 64:(g + 1) * 64, :].T
        return out.astype(bf16)

    def blockdiag(w):
        # w: [2, 256, H] -> [128, 512]; per new-gate [128,128] block-diag
        out = np.zeros((128, 512), np.float32)
        for gi, g in enumerate(_GPERM):
            out[0:_H, gi * 128:gi * 128 + 64] = \
                w[0, g * 64:(g + 1) * 64, :].T
            out[_H:128, gi * 128 + 64:(gi + 1) * 128] = \
                w[1, g * 64:(g + 1) * 64, :].T
        return out.astype(bf16)

    def biases(bi, bh):
        b = bi + bh  # [2, 256]
        out = np.empty((128, 4), np.float32)
        for gi, g in enumerate(_GPERM):
            out[0:_H, gi] = b[0, g * 64:(g + 1) * 64]
            out[_H:128, gi] = b[1, g * 64:(g + 1) * 64]
        return out

    shared = {
        "wi0T": np.ascontiguousarray(gate_stack_T(wih0)),
        "wi1T": np.ascontiguousarray(gate_stack_T(wih1)),
        "wh0bd": blockdiag(whh0),
        "wh1bd": blockdiag(whh1),
        "bias0": biases(bih0, bhh0),
        "bias1": biases(bih1, bhh1),
    }
    in_maps = []
    for k in range(_NCORES):
        rows = [x1[2 * k, :T], x1[2 * k + 1, :T], x2[2 * k, :T],
                x2[2 * k + 1, :T]]
        arr = np.stack(rows, axis=2)            # [T, D, S]
        xc = np.ascontiguousarray(
            arr.transpose(1, 0, 2).reshape(_D, T * _S)).astype(bf16)
        xr = np.ascontiguousarray(
            arr[::-1].transpose(1, 0, 2).reshape(_D, T * _S)).astype(bf16)
        m = dict(shared)
        m["x_cat"] = xc
        m["x_rev"] = xr
        m["u2"] = np.ascontiguousarray(u[2 * k:2 * k + 2, :T])
        in_maps.append(m)
    return in_maps


def _run(inputs, T=_T, trace=False, dbg=False):
    from concourse import bass_utils
    key = (T, dbg)
    if key not in _cache:
        _cache[key] = _build(T, dbg)
    nc = _cache[key]
    in_maps = _host_prep(T=T, **inputs)
    res = bass_utils.run_bass_kernel_spmd(
        nc, in_maps, core_ids=list(range(_NCORES)), trace=trace)
    N = _NCORES * _BS * T
    R = sum(float(r["outp"][:, 0].sum()) for r in res.results)
    Draw = sum(float(r["outp"][0, 2]) for r in res.results)
    colsum = np.zeros(N, np.float64)
    for r in res.results:
        colsum += np.asarray(r["colsum"][0], np.float64)
    C = float(np.log(colsum).sum())
    Dg = Draw / _TEMP
    loss = -((Dg - R) / N + (Dg - C) / N)
    return np.float32(loss), res


def kernel(**inputs):
    loss, _ = _run(inputs)
    return np.asarray(loss, dtype=np.float32)



# revision 2
# speedup vs baseline: 1.0483x; 1.0483x over previous
"""Trainium2 Bass kernel for nn_MIPS_74904229642848 (v3).

Pipeline (8 NeuronCores, SPMD, batch-sharded 2 rows/core, S=4 streams/core):
  1. 2-layer bidirectional LSTM, all-bf16 matmuls. Per step: one identity
     matmul injects the precomputed input-gate terms into PSUM (chunked so
     each recurrence starts after its first xg chunk), four bf16 block-diag
     recurrence matmuls accumulate on top. The g-gate weights/bias are
     pre-scaled by 2 on the host so ONE sigmoid over all four gates covers
     the tanh as well (tanh(g) = 2*sigmoid(2g)-1); the cell update is two
     fused scalar_tensor_tensor ops on DVE (m1' = (u-.5)*si;
     c = 2*m1' + m2) with m2 = sf*c on GpSimd, c in SBUF, tanh(c) on ACT,
     h = so*tanh(c) on DVE into a triple-buffered Hst.
  2. L2 normalization via ln/exp rsqrt (no Newton), fused scale+pack.
  3. Windowed index sampling of z2 via indirect DMA (bf16).
  4. AllGather of the B-side embeddings only (bf16).
  5. One-pass logits sweep: bf16 sim matmul blocks, exp (ACT, accum_out
     gives row sums), ones-matmul accumulates column sums in PSUM across
     row blocks. Row-lse finished on device; per-core column-sum partials
     shipped to the host, which does the final ln+sum combine.
"""

import numpy as np

_D, _E, _H, _B, _W = 64, 128, 64, 16, 3
_T = 512
_TEMP = 0.05
_NCORES = 8
_BS = _B // _NCORES          # batch rows per core
_S = 2 * _BS                 # streams per core: (x1,b0),(x1,b1),(x2,b0),(x2,b1)
_GF = 4 * _S                 # gate-block width per step

# torch gate order i,f,g,o -> kernel order i,f,o,g (tanh block last)
_GPERM = [0, 1, 3, 2]
_GTANH = 3                   # index of the g gate in kernel order

_cache = {}


def _build(T, dbg=False):
    import concourse.bass as bass
    import concourse.mybir as mybir
    import concourse.tile as tile
    from concourse import bacc, library_config
    from concourse.masks import make_identity

    f32 = mybir.dt.float32
    bf16 = mybir.dt.bfloat16
    i32 = mybir.dt.int32
    AF = mybir.ActivationFunctionType
    OP = mybir.AluOpType

    S = _S
    GF = _GF
    TS = T * S
    NLOC = _BS * T
    NGLOB = _NCORES * NLOC
    FC = min(512, TS)            # xg free chunk
    NFC = TS // FC
    TCH = FC // S                # timesteps per xg chunk
    CC = min(512, NGLOB)         # logits col chunk
    NCC = NGLOB // CC
    NRC = (NLOC + 127) // 128    # logits row chunks (M=128)
    NTC = T // 128               # transpose chunks per stream
    NUC = T // 128

    nc = bacc.Bacc("TRN2", target_bir_lowering=False, debug=False,
                   num_devices=_NCORES)

    # ---- I/O ----
    x_in = nc.dram_tensor("x_cat", [_D, TS], bf16, kind="ExternalInput")
    xr_in = nc.dram_tensor("x_rev", [_D, TS], bf16, kind="ExternalInput")
    # u and the t-ramp in ap_gather's wrapped layout:
    # col = b*32 + s, row p; value index t = s*16 + p%16
    NW = T // 16
    u16_in = nc.dram_tensor("u16", [128, _BS * NW], f32,
                            kind="ExternalInput")
    tw_in = nc.dram_tensor("tw", [128, _BS * NW], f32,
                           kind="ExternalInput")
    wi0_in = nc.dram_tensor("wi0T", [_D, 512], bf16, kind="ExternalInput")
    wi1_in = nc.dram_tensor("wi1T", [_E, 512], bf16, kind="ExternalInput")
    wh0_in = nc.dram_tensor("wh0bd", [_E, 512], bf16, kind="ExternalInput")
    wh1_in = nc.dram_tensor("wh1bd", [_E, 512], bf16, kind="ExternalInput")
    b0_in = nc.dram_tensor("bias0", [_E, 4], f32, kind="ExternalInput")
    b1_in = nc.dram_tensor("bias1", [_E, 4], f32, kind="ExternalInput")
    out_d = nc.dram_tensor("outp", [128, 4], f32, kind="ExternalOutput")
    cs_d = nc.dram_tensor("colsum", [1, NGLOB], f32, kind="ExternalOutput")
    if dbg:
        anb_d = nc.dram_tensor("anb", [128, 4 * NLOC], f32,
                               kind="ExternalOutput")

    # DRAM scratch for the AllGather
    ag_in_d = nc.dram_tensor("ag_in", [128, NLOC], bf16)
    ag_out_d = nc.dram_tensor("ag_out", [_NCORES * 128, NLOC], bf16,
                              addr_space="Shared")

    with tile.TileContext(nc) as tc:
        with (
            tc.tile_pool(name="consts", bufs=1) as consts,
            tc.tile_pool(name="bigbuf", bufs=1) as bigbuf,
            tc.tile_pool(name="state", bufs=1) as state,
            tc.tile_pool(name="small", bufs=3) as small,
            tc.tile_pool(name="nrm", bufs=1) as nrm,
            tc.tile_pool(name="sg", bufs=4) as sgp,
        ):
            dma = nc.sync.dma_start

            # ---------- load constants / inputs ----------
            x_sb = bigbuf.tile([_D, TS], bf16, tag="x")
            xr_sb = bigbuf.tile([_D, TS], bf16, tag="xr")
            dma(out=x_sb[:], in_=x_in.ap())
            dma(out=xr_sb[:], in_=xr_in.ap())
            wi0_sb = consts.tile([_D, 512], bf16)
            wi1_sb = consts.tile([_E, 512], bf16)
            wh0_sb = consts.tile([_E, 512], bf16)
            wh1_sb = consts.tile([_E, 512], bf16)
            b0_sb = consts.tile([_E, 4], f32)
            b1_sb = consts.tile([_E, 4], f32)
            for sb, di in ((wi0_sb, wi0_in), (wi1_sb, wi1_in),
                           (wh0_sb, wh0_in), (wh1_sb, wh1_in),
                           (b0_sb, b0_in), (b1_sb, b1_in)):
                dma(out=sb[:], in_=di.ap())
            ident = consts.tile([128, 128], bf16)
            make_identity(nc, ident[:])
            ones_col = consts.tile([128, 1], bf16)
            nc.vector.memset(ones_col[:], 1.0)
            ones_row = consts.tile([1, 128], f32)
            nc.vector.memset(ones_row[:], 1.0)

            # ---------- window indices on device (only needs u) ----------
            # computed in ap_gather's wrapped layout [128, b*32+s]
            NWB = _BS * NW
            u16 = consts.tile([128, NWB], f32)
            tf = consts.tile([128, NWB], f32)
            dma(out=u16[:], in_=u16_in.ap())
            dma(out=tf[:], in_=tw_in.ap())
            ks = small.tile([128, NWB], f32, tag="ks")
            nc.vector.tensor_scalar_mul(ks[:], tf[:], 1.0 / (T - 1))
            # the oracle's (k*s).astype(int32) rounds-to-nearest on neuron,
            # so the carry fires at 0.5 rather than 1.0
            nc.vector.tensor_scalar(ks[:], ks[:], 0.5, None, OP.is_ge)
            nc.vector.tensor_add(ks[:], tf[:], ks[:])          # center
            lo = small.tile([128, NWB], f32, tag="lo")
            nc.vector.tensor_scalar(lo[:], ks[:], -float(_W), 0.0,
                                    OP.add, OP.max)
            hi = small.tile([128, NWB], f32, tag="hi")
            nc.vector.tensor_scalar(hi[:], ks[:], float(_W), float(T),
                                    OP.add, OP.min)
            cnt = small.tile([128, NWB], f32, tag="cnt")
            nc.vector.tensor_sub(cnt[:], hi[:], lo[:])
            pr = small.tile([128, NWB], f32, tag="pr")
            nc.vector.tensor_mul(pr[:], u16[:], cnt[:])
            # floor(pr) for pr in [0, 2W+1): sum of is_ge thresholds
            fr = small.tile([128, NWB], f32, tag="fr")
            nc.vector.memset(fr[:], 0.0)
            for kth in range(1, 2 * _W + 1):
                nc.vector.scalar_tensor_tensor(
                    fr[:], pr[:], float(kth), fr[:], OP.is_ge, OP.add)
            nc.vector.tensor_add(lo[:], lo[:], fr[:])          # i2 (pre-min)
            nc.vector.tensor_scalar_add(hi[:], hi[:], -1.0)
            nc.vector.tensor_tensor(lo[:], lo[:], hi[:], op=OP.min)
            idx16 = consts.tile([128, NWB], mybir.dt.int16)
            nc.vector.tensor_copy(idx16[:], lo[:])

            # ---------- LSTM ----------
            def xg_precompute(ps_big, wiT_sb, K, srcs_f, srcs_b, bias_sb,
                              XGs):
                # XG layout: [128, (t, gate, s)] with gf = GF per step,
                # chunked into NFC tiles so the recurrence can start as
                # soon as chunk 0 is written
                for ch in range(NFC):
                    XG3 = XGs[ch][:].rearrange("p (t gf) -> p t gf", gf=GF)
                    for g in range(4):
                        ps = ps_big.tile([128, FC], f32, tag="xgps")
                        nc.tensor.matmul(
                            out=ps[0:_H, :],
                            lhsT=wiT_sb[0:K, g * 64:(g + 1) * 64],
                            rhs=srcs_f[ch],
                            start=True, stop=True)
                        nc.tensor.matmul(
                            out=ps[_H:128, :],
                            lhsT=wiT_sb[0:K, 256 + g * 64:256 + (g + 1) * 64],
                            rhs=srcs_b[ch],
                            start=True, stop=True)
                        dst = XG3[:, :, g * S:(g + 1) * S]
                        nc.scalar.activation(
                            dst,
                            ps[:].rearrange("p (t s) -> p t s", s=S),
                            AF.Identity, bias=bias_sb[:, g:g + 1])

            def recurrence(ps_gate, wh_sb, XGs, Ht, Hr=None, lyr=0):
                # Cell state via tensor_tensor_scan: we track ct = c/2, so
                #   ct_t = sf * ct_{t-1} + m1',  m1' = (u-0.5)*si
                #        (= [sf*c + si*tanh(g)] / 2 since tanh(g) = 2u-1)
                #   tanh(c) = Tanh(2*ct)  (ACT scale=2)
                # The scan's free layout interleaves a reset slot (d0=0,
                # d1=ct_prev) and an update slot (d0=sf, d1=m1') per stream.
                # sigma writes all gates strided into sgx (odd cols); even
                # cols stay 0 so sgx[8:16] is [0 sf 0 sf ...] = the scan d0.
                # R tiles: scan(t) writes [e0 n0 e1 n1 ...] into R[t%2][0:8]
                # (n_s = new ct at col 2s+1); m1'(t+1) then lands at cols
                # {2,4,6,8} of the same tile, so R[t%2][1:9] is the next d1.
                Hsts = [state.tile([128, S], bf16, tag=f"hst{lyr}{i}",
                                   name=f"hst{lyr}{i}") for i in range(3)]
                Rs = [state.tile([128, 2 * S + 2], f32, tag=f"r{lyr}{i}",
                                 name=f"r{lyr}{i}") for i in range(2)]
                Tc = [state.tile([128, S], bf16, tag=f"tc{lyr}{i}",
                                 name=f"tc{lyr}{i}") for i in range(2)]
                Sgx = [state.tile([128, 2 * GF], f32, tag=f"sg{lyr}{i}",
                                  name=f"sg{lyr}{i}") for i in range(2)]
                for i in range(3):
                    nc.vector.memset(Hsts[i][:], 0.0)
                for i in range(2):
                    nc.vector.memset(Rs[i][:], 0.0)
                    nc.vector.memset(Sgx[i][:], 0.0)

                def sview(sgx, g):
                    # [128, S, 1] view of gate g's columns {2*(g*S+s)+1}
                    v = sgx[:].rearrange("p (c z) -> p c z", z=2)
                    return v[:, g * S:(g + 1) * S, 1:2]

                def emit_copies(t, Hst):
                    rt = T - 1 - t
                    nc.gpsimd.tensor_copy(Ht[0:_H, t * S:(t + 1) * S],
                                          Hst[0:_H, :])
                    nc.gpsimd.tensor_copy(Ht[_H:128, rt * S:(rt + 1) * S],
                                          Hst[_H:128, :])
                    if Hr is not None:
                        nc.gpsimd.tensor_copy(Hr[0:_H, rt * S:(rt + 1) * S],
                                              Hst[0:_H, :])
                        nc.gpsimd.tensor_copy(Hr[_H:128, t * S:(t + 1) * S],
                                              Hst[_H:128, :])

                for t in range(T):
                    Hprev = Hsts[(t + 2) % 3]
                    Hst = Hsts[t % 3]
                    Ra = Rs[t % 2]
                    Rb = Rs[(t + 1) % 2]
                    tc = Tc[t % 2]
                    sgx = Sgx[t % 2]
                    gb = ps_gate.tile([128, GF], f32, tag="gates")
                    xgsl = XGs[t // TCH][:, (t % TCH) * GF:
                                         (t % TCH + 1) * GF]
                    # inject xg via identity matmul (clears has_written)
                    nc.tensor.matmul(out=gb[:], lhsT=ident[:],
                                     rhs=xgsl,
                                     start=True, stop=False,
                                     skip_group_check=True)
                    for g in range(4):
                        nc.tensor.matmul(
                            out=gb[:, g * S:(g + 1) * S],
                            lhsT=wh_sb[:, g * 128:(g + 1) * 128],
                            rhs=Hprev[:],
                            start=False, stop=(g == 3),
                            skip_group_check=True)
                    # one sigmoid covers all gates (g-gate weights x2 on
                    # host, so its col holds u = sigmoid(2g)); output is
                    # strided into odd cols of sgx
                    gb3 = gb[:].rearrange("p (c z) -> p c z", z=1)
                    sgo = sgx[:].rearrange("p (c z) -> p c z", z=2)[:, :, 1:2]
                    nc.scalar.activation(sgo, gb3, AF.Sigmoid)
                    # m1' = (u - 0.5) * si into Rb cols {2,4,6,8}
                    m1o = Rb[:, 2:2 * S + 2].rearrange(
                        "p (s z) -> p s z", z=2)[:, :, 0:1]
                    nc.vector.scalar_tensor_tensor(
                        m1o, sview(sgx, 3), 0.5, sview(sgx, 0),
                        OP.subtract, OP.mult)
                    # ct scan: d0 = [0 sf 0 sf ...], d1 = [ct_prev m1' ...]
                    nc.vector.tensor_tensor_scan(
                        Ra[:, 0:2 * S], sgx[:, 2 * S:4 * S],
                        Rb[:, 1:2 * S + 1], 0.0, OP.mult, OP.add)
                    # tanh(c) = Tanh(2*ct) from odd cols of Ra
                    tanh_in = Ra[:, 1:2 * S + 1].rearrange(
                        "p (s z) -> p s z", z=2)[:, :, 0:1]
                    tc3 = tc[:].rearrange("p (s z) -> p s z", z=1)
                    nc.scalar.activation(tc3, tanh_in, AF.Tanh, scale=2.0)
                    hst3 = Hst[:].rearrange("p (s z) -> p s z", z=1)
                    nc.vector.tensor_tensor(hst3, sview(sgx, 2), tc3,
                                            op=OP.mult)
                    emit_copies(t, Hst)

            H0t = bigbuf.tile([128, TS], bf16, tag="h0t")
            H1t = bigbuf.tile([128, TS], bf16, tag="h1t")
            with (
                tc.tile_pool(name="ps_big", bufs=2, space="PSUM") as ps_big,
                tc.tile_pool(name="ps_gate", bufs=3, space="PSUM") as ps_gate,
            ):
                XGs = [bigbuf.tile([128, TCH * GF], bf16, tag=f"xg{ch}",
                                   name=f"xg{ch}")
                       for ch in range(NFC)]
                H0r = bigbuf.tile([128, TS], bf16, tag="h0r")
                srcs_f = [x_sb[:, ch * FC:(ch + 1) * FC] for ch in range(NFC)]
                srcs_b = [xr_sb[:, ch * FC:(ch + 1) * FC] for ch in range(NFC)]
                xg_precompute(ps_big, wi0_sb, _D, srcs_f, srcs_b, b0_sb, XGs)
                recurrence(ps_gate, wh0_sb, XGs, H0t, H0r, lyr=0)

                # layer 2: fwd reads H0t, bwd reads the reversed copy H0r
                # (negative-stride APs are rejected by the BIR verifier)
                srcs_f = [H0t[:, ch * FC:(ch + 1) * FC] for ch in range(NFC)]
                srcs_b = [H0r[:, ch * FC:(ch + 1) * FC] for ch in range(NFC)]
                xg_precompute(ps_big, wi1_sb, _E, srcs_f, srcs_b, b1_sb, XGs)
                recurrence(ps_gate, wh1_sb, XGs, H1t, lyr=1)

            # ---------- normalize + pack An (z1) / Z2 ----------
            An = bigbuf.tile([128, NLOC], bf16, tag="an")
            Z2f = bigbuf.tile([128, NLOC], f32, tag="z2")
            Bn = bigbuf.tile([128, NLOC], bf16, tag="bn")
            Bgf = bigbuf.tile([128, NLOC], f32, tag="bgf")
            H13 = H1t[:].rearrange("p (t s) -> p t s", s=S)
            n2 = nrm.tile([1, TS], f32, tag="n2")
            sq = bigbuf.tile([128, FC], bf16, tag="sq")
            with (
                tc.tile_pool(name="ps_nrm", bufs=4, space="PSUM") as ps_nrm,
                tc.tile_pool(name="ps_row", bufs=2, space="PSUM") as ps_row,
            ):
                for ch in range(NFC):
                    nc.vector.tensor_mul(sq[:],
                                         H1t[:, ch * FC:(ch + 1) * FC],
                                         H1t[:, ch * FC:(ch + 1) * FC])
                    psn = ps_row.tile([1, FC], f32, tag="psn")
                    nc.tensor.matmul(out=psn[:], lhsT=ones_col[:],
                                     rhs=sq[:], start=True, stop=True)
                    nc.vector.tensor_copy(n2[:, ch * FC:(ch + 1) * FC],
                                          psn[:])
                nc.vector.tensor_scalar_max(n2[:], n2[:], 1e-24)
                lnb = nrm.tile([1, TS], f32, tag="lnb")
                nc.scalar.activation(lnb[:], n2[:], AF.Ln)
                rin = nrm.tile([1, TS], f32, tag="rin")
                nc.scalar.activation(rin[:], lnb[:], AF.Exp, scale=-0.5)
                # pack Z2 first: the gather + chunked AllGather start as
                # early as possible; An is packed during the collectives
                def pack(streams, ch, psb):
                    ps3 = psb[:].rearrange("p (t s) -> p t s", s=S)
                    h3 = H13[:, ch * TCH:(ch + 1) * TCH, :]
                    for s in streams:
                        if s < _BS:
                            dst = An[:, s * T + ch * TCH:
                                     s * T + (ch + 1) * TCH]
                        else:
                            b = s - _BS
                            dst = Z2f[:, b * T + ch * TCH:
                                      b * T + (ch + 1) * TCH]
                        nc.vector.tensor_tensor(
                            dst,
                            h3[:, :, s:s + 1].rearrange("p t o -> p (t o)"),
                            ps3[:, :, s:s + 1].rearrange("p t o -> p (t o)"),
                            op=OP.mult)

                def mkpsb(ch):
                    psb = ps_nrm.tile([128, FC], f32, tag="nps")
                    nc.tensor.matmul(out=psb[:], lhsT=ones_row[:],
                                     rhs=rin[:, ch * FC:(ch + 1) * FC],
                                     start=True, stop=True)
                    return psb
                psbs = []
                for ch in range(NFC):
                    psb = mkpsb(ch)
                    psbs.append(psb)
                    pack(range(_BS, S), ch, psb)

                # ---------- gather z2 -> Bn + AllGather ----------
                nc.gpsimd.load_library(library_config.ap_gather)
                for b in range(_BS):
                    nc.gpsimd.ap_gather(
                        Bgf[:, b * T:(b + 1) * T],
                        Z2f[:, b * T:(b + 1) * T],
                        idx16[:, b * NW:(b + 1) * NW],
                        channels=128, num_elems=T, d=1, num_idxs=T)
                    nc.vector.tensor_copy(Bn[:, b * T:(b + 1) * T],
                                          Bgf[:, b * T:(b + 1) * T])
                dma(out=ag_in_d.ap(), in_=Bn[:])
                nc.gpsimd.collective_compute(
                    "AllGather", OP.bypass,
                    replica_groups=[list(range(_NCORES))],
                    ins=[ag_in_d.ap().opt()],
                    outs=[ag_out_d.ap().opt()])

                # pack An while the collectives run
                for ch in range(NFC):
                    pack(range(_BS), ch, psbs[ch])

            Ball = bigbuf.tile([128, NGLOB], bf16, tag="ball")
            for jj in range(_NCORES):
                dma(out=Ball[:, jj * NLOC:(jj + 1) * NLOC],
                    in_=ag_out_d.ap()[jj * 128:(jj + 1) * 128, :])

            # ---------- output partials ----------
            outp = consts.tile([128, 4], f32)
            nc.vector.memset(outp[:], 0.0)

            with (
                tc.tile_pool(name="ps_d", bufs=1, space="PSUM") as ps_d,
                tc.tile_pool(name="ps_s", bufs=2, space="PSUM") as ps_s,
                tc.tile_pool(name="ps_c", bufs=1, space="PSUM") as ps_c,
            ):
                # diag: sum_i <An_i, Bn_i>
                dg = bigbuf.tile([128, NLOC], bf16, tag="dg")
                nc.vector.tensor_mul(dg[:], An[:], Bn[:])
                ndc = (NLOC + 511) // 512
                psd = ps_d.tile([1, 512], f32, tag="psd")
                for ch in range(ndc):
                    nc.tensor.matmul(out=psd[:], lhsT=ones_col[:],
                                     rhs=dg[:, ch * 512:(ch + 1) * 512],
                                     start=(ch == 0), stop=(ch == ndc - 1))
                nc.vector.reduce_sum(outp[0:1, 2:3], psd[:],
                                     axis=mybir.AxisListType.X)

                # one-pass row+col logsumexp sweep ([128,1024] exp
                # blocks; row sums on DVE, col sums via ones-matmuls)
                CP = 2 * CC
                NCP = NGLOB // CP
                rows = bigbuf.tile([128, NRC * NCP], f32, tag="rows")
                csum = nrm.tile([1, NGLOB], f32, tag="csum")
                for cb in range(NCP):
                    csp0 = ps_c.tile([1, CC], f32, tag="csp0")
                    csp1 = ps_c.tile([1, CC], f32, tag="csp1")
                    for rc in range(NRC):
                        ps = ps_s.tile([128, CP], f32, tag="sps")
                        nc.tensor.matmul(
                            out=ps[:, 0:CC],
                            lhsT=An[:, rc * 128:(rc + 1) * 128],
                            rhs=Ball[:, cb * CP:cb * CP + CC],
                            start=True, stop=True)
                        nc.tensor.matmul(
                            out=ps[:, CC:CP],
                            lhsT=An[:, rc * 128:(rc + 1) * 128],
                            rhs=Ball[:, cb * CP + CC:(cb + 1) * CP],
                            start=True, stop=True)
                        eb = sgp.tile([128, CP], bf16, tag="eb")
                        nc.scalar.activation(
                            eb[:], ps[:], AF.Exp, scale=1.0 / _TEMP)
                        nc.vector.reduce_sum(
                            rows[:, rc * NCP + cb:rc * NCP + cb + 1],
                            eb[:], axis=mybir.AxisListType.X)
                        nc.tensor.matmul(
                            out=csp0[:], lhsT=ones_col[:], rhs=eb[:, 0:CC],
                            start=(rc == 0), stop=(rc == NRC - 1))
                        nc.tensor.matmul(
                            out=csp1[:], lhsT=ones_col[:], rhs=eb[:, CC:CP],
                            start=(rc == 0), stop=(rc == NRC - 1))
                    nc.vector.tensor_copy(csum[:, cb * CP:cb * CP + CC],
                                          csp0[:])
                    nc.vector.tensor_copy(
                        csum[:, cb * CP + CC:(cb + 1) * CP], csp1[:])

                tot = small.tile([128, NRC], f32, tag="tot")
                for rc in range(NRC):
                    nc.vector.reduce_sum(tot[:, rc:rc + 1],
                                         rows[:, rc * NCP:(rc + 1) * NCP],
                                         axis=mybir.AxisListType.X)
                lse = small.tile([128, NRC], f32, tag="lse")
                nc.scalar.activation(lse[:], tot[:], AF.Ln)
                nc.vector.reduce_sum(outp[:, 0:1], lse[:],
                                     axis=mybir.AxisListType.X)

            dma(out=out_d.ap(), in_=outp[:])
            dma(out=cs_d.ap(), in_=csum[:])
            if dbg:
                anb32 = bigbuf.tile([128, NLOC], f32, tag="anb32")
                for i, src in enumerate((An, Bn, Z2f)):
                    nc.vector.tensor_copy(anb32[:], src[:])
                    dma(out=anb_d.ap()[:, i * NLOC:(i + 1) * NLOC],
                        in_=anb32[:])
                nc.vector.tensor_copy(anb32[:], H1t[:, 0:NLOC])
                dma(out=anb_d.ap()[:, 3 * NLOC:4 * NLOC], in_=anb32[:])

    nc.compile()
    return nc


def _host_prep(x1, x2, u, wih0, whh0, bih0, bhh0, wih1, whh1, bih1, bhh1, T):
    """Build per-core input maps (all host work is pure data layout)."""
    import ml_dtypes
    bf16 = ml_dtypes.bfloat16

    # g-gate (kernel position _GTANH) weights/bias are pre-scaled by 2 so
    # a single sigmoid gives u = sigmoid(2g), tanh(g) = 2u - 1
    def gate_stack_T(w):
        # w: [2, 256, K] -> [K, 512]; cols = dir*256 + gperm_gate*64 + j
        K = w.shape[2]
        out = np.empty((K, 512), np.float32)
        for d in range(2):
            for gi, g in enumerate(_GPERM):
                sc = 2.0 if gi == _GTANH else 1.0
                out[:, d * 256 + gi * 64:d * 256 + (gi + 1) * 64] = \
                    sc * w[d, g * 64:(g + 1) * 64, :].T
        return out.astype(bf16)

    def blockdiag(w):
        # w: [2, 256, H] -> [128, 512]; per new-gate [128,128] block-diag
        out = np.zeros((128, 512), np.float32)
        for gi, g in enumerate(_GPERM):
            sc = 2.0 if gi == _GTANH else 1.0
            out[0:_H, gi * 128:gi * 128 + 64] = \
                sc * w[0, g * 64:(g + 1) * 64, :].T
            out[_H:128, gi * 128 + 64:(gi + 1) * 128] = \
                sc * w[1, g * 64:(g + 1) * 64, :].T
        return out.astype(bf16)

    def biases(bi, bh):
        b = bi + bh  # [2, 256]
        out = np.empty((128, 4), np.float32)
        for gi, g in enumerate(_GPERM):
            sc = 2.0 if gi == _GTANH else 1.0
            out[0:_H, gi] = sc * b[0, g * 64:(g + 1) * 64]
            out[_H:128, gi] = sc * b[1, g * 64:(g + 1) * 64]
        return out

    # t-ramp in ap_gather's wrapped layout: tw[p, b*NW+s] = s*16 + p%16
    NW = T // 16
    ramp = (np.arange(NW)[:, None] * 16 +
            np.arange(16)[None, :]).astype(np.float32)   # [NW, 16]
    twt = np.tile(ramp.T, (8, _BS))                      # [128, BS*NW]

    def wrap_u(uc):
        # uc: [BS, T] -> [128, BS*NW] wrapped + replicated per 16-part group
        cols = [np.tile(uc[b].reshape(NW, 16).T, (8, 1)) for b in range(_BS)]
        return np.ascontiguousarray(np.concatenate(cols, axis=1))

    shared = {
        "wi0T": np.ascontiguousarray(gate_stack_T(wih0)),
        "wi1T": np.ascontiguousarray(gate_stack_T(wih1)),
        "wh0bd": blockdiag(whh0),
        "wh1bd": blockdiag(whh1),
        "bias0": biases(bih0, bhh0),
        "bias1": biases(bih1, bhh1),
        "tw": np.ascontiguousarray(twt),
    }
    in_maps = []
    for k in range(_NCORES):
        rows = [x1[2 * k, :T], x1[2 * k + 1, :T], x2[2 * k, :T],
                x2[2 * k + 1, :T]]
        arr = np.stack(rows, axis=2)            # [T, D, S]
        xc = np.ascontiguousarray(
            arr.transpose(1, 0, 2).reshape(_D, T * _S)).astype(bf16)
        xr = np.ascontiguousarray(
            arr[::-1].transpose(1, 0, 2).reshape(_D, T * _S)).astype(bf16)
        m = dict(shared)
        m["x_cat"] = xc
        m["x_rev"] = xr
        m["u16"] = wrap_u(u[2 * k:2 * k + 2, :T])
        in_maps.append(m)
    return in_maps


def _run(inputs, T=_T, trace=False, dbg=False):
    from concourse import bass_utils
    key = (T, dbg)
    if key not in _cache:
        _cache[key] = _build(T, dbg)
    nc = _cache[key]
    in_maps = _host_prep(T=T, **inputs)
    res = bass_utils.run_bass_kernel_spmd(
        nc, in_maps, core_ids=list(range(_NCORES)), trace=trace)
    N = _NCORES * _BS * T
    R = sum(float(r["outp"][:, 0].sum()) for r in res.results)
    Draw = sum(float(r["outp"][0, 2]) for r in res.results)
    colsum = np.zeros(N, np.float64)
    for r in res.results:
        colsum += np.asarray(r["colsum"][0], np.float64)
    C = float(np.log(colsum).sum())
    Dg = Draw / _TEMP
    loss = -((Dg - R) / N + (Dg - C) / N)
    return np.float32(loss), res


def kernel(**inputs):
    loss, _ = _run(inputs)
    return np.asarray(loss, dtype=np.float32)



# revision 3
# speedup vs baseline: 1.0493x; 1.0009x over previous
"""Trainium2 Bass kernel for nn_MIPS_74904229642848 (v3).

Pipeline (8 NeuronCores, SPMD, batch-sharded 2 rows/core, S=4 streams/core):
  1. 2-layer bidirectional LSTM, all-bf16 matmuls. Per step: one identity
     matmul injects the precomputed input-gate terms into PSUM (chunked so
     each recurrence starts after its first xg chunk), four bf16 block-diag
     recurrence matmuls accumulate on top. The g-gate weights/bias are
     pre-scaled by 2 on the host so ONE sigmoid over all four gates covers
     the tanh as well (tanh(g) = 2*sigmoid(2g)-1); the cell update is two
     fused scalar_tensor_tensor ops on DVE (m1' = (u-.5)*si;
     c = 2*m1' + m2) with m2 = sf*c on GpSimd, c in SBUF, tanh(c) on ACT,
     h = so*tanh(c) on DVE into a triple-buffered Hst.
  2. L2 normalization via ln/exp rsqrt (no Newton), fused scale+pack.
  3. Windowed index sampling of z2 via indirect DMA (bf16).
  4. AllGather of the B-side embeddings only (bf16).
  5. One-pass logits sweep: bf16 sim matmul blocks, exp (ACT, accum_out
     gives row sums), ones-matmul accumulates column sums in PSUM across
     row blocks. Row-lse finished on device; per-core column-sum partials
     shipped to the host, which does the final ln+sum combine.
"""

import numpy as np

_D, _E, _H, _B, _W = 64, 128, 64, 16, 3
_T = 512
_TEMP = 0.05
_NCORES = 8
_BS = _B // _NCORES          # batch rows per core
_S = 2 * _BS                 # streams per core: (x1,b0),(x1,b1),(x2,b0),(x2,b1)
_GF = 4 * _S                 # gate-block width per step

# torch gate order i,f,g,o -> kernel order o,i,f,g (tanh block last)
_GPERM = [3, 0, 1, 2]
_GTANH = 3                   # index of the g gate in kernel order

_cache = {}


def _build(T, dbg=False):
    import concourse.bass as bass
    import concourse.mybir as mybir
    import concourse.tile as tile
    from concourse import bacc, library_config
    from concourse.masks import make_identity

    f32 = mybir.dt.float32
    bf16 = mybir.dt.bfloat16
    i32 = mybir.dt.int32
    AF = mybir.ActivationFunctionType
    OP = mybir.AluOpType

    S = _S
    GF = _GF
    TS = T * S
    NLOC = _BS * T
    NGLOB = _NCORES * NLOC
    FC = min(512, TS)            # xg free chunk
    NFC = TS // FC
    TCH = FC // S                # timesteps per xg chunk
    CC = min(512, NGLOB)         # logits col chunk
    NCC = NGLOB // CC
    NRC = (NLOC + 127) // 128    # logits row chunks (M=128)
    NTC = T // 128               # transpose chunks per stream
    NUC = T // 128

    nc = bacc.Bacc("TRN2", target_bir_lowering=False, debug=False,
                   num_devices=_NCORES)

    # ---- I/O ----
    x_in = nc.dram_tensor("x_cat", [_D, TS], bf16, kind="ExternalInput")
    xr_in = nc.dram_tensor("x_rev", [_D, TS], bf16, kind="ExternalInput")
    # u and the t-ramp in ap_gather's wrapped layout:
    # col = b*32 + s, row p; value index t = s*16 + p%16
    NW = T // 16
    u16_in = nc.dram_tensor("u16", [128, _BS * NW], f32,
                            kind="ExternalInput")
    tw_in = nc.dram_tensor("tw", [128, _BS * NW], f32,
                           kind="ExternalInput")
    wi0_in = nc.dram_tensor("wi0T", [_D, 512], bf16, kind="ExternalInput")
    wi1_in = nc.dram_tensor("wi1T", [_E, 512], bf16, kind="ExternalInput")
    wh0_in = nc.dram_tensor("wh0bd", [_E, 512], bf16, kind="ExternalInput")
    wh1_in = nc.dram_tensor("wh1bd", [_E, 512], bf16, kind="ExternalInput")
    b0_in = nc.dram_tensor("bias0", [_E, 4], f32, kind="ExternalInput")
    b1_in = nc.dram_tensor("bias1", [_E, 4], f32, kind="ExternalInput")
    out_d = nc.dram_tensor("outp", [128, 4], f32, kind="ExternalOutput")
    cs_d = nc.dram_tensor("colsum", [1, NGLOB], f32, kind="ExternalOutput")
    if dbg:
        anb_d = nc.dram_tensor("anb", [128, 4 * NLOC], f32,
                               kind="ExternalOutput")

    # DRAM scratch for the AllGather
    ag_in_d = nc.dram_tensor("ag_in", [128, NLOC], bf16)
    ag_out_d = nc.dram_tensor("ag_out", [_NCORES * 128, NLOC], bf16,
                              addr_space="Shared")

    with tile.TileContext(nc) as tc:
        with (
            tc.tile_pool(name="consts", bufs=1) as consts,
            tc.tile_pool(name="bigbuf", bufs=1) as bigbuf,
            tc.tile_pool(name="state", bufs=1) as state,
            tc.tile_pool(name="small", bufs=3) as small,
            tc.tile_pool(name="nrm", bufs=1) as nrm,
            tc.tile_pool(name="sg", bufs=4) as sgp,
        ):
            dma = nc.sync.dma_start

            # ---------- load constants / inputs ----------
            x_sb = bigbuf.tile([_D, TS], bf16, tag="x")
            xr_sb = bigbuf.tile([_D, TS], bf16, tag="xr")
            dma(out=x_sb[:], in_=x_in.ap())
            dma(out=xr_sb[:], in_=xr_in.ap())
            wi0_sb = consts.tile([_D, 512], bf16)
            wi1_sb = consts.tile([_E, 512], bf16)
            wh0_sb = consts.tile([_E, 512], bf16)
            wh1_sb = consts.tile([_E, 512], bf16)
            b0_sb = consts.tile([_E, 4], f32)
            b1_sb = consts.tile([_E, 4], f32)
            for sb, di in ((wi0_sb, wi0_in), (wi1_sb, wi1_in),
                           (wh0_sb, wh0_in), (wh1_sb, wh1_in),
                           (b0_sb, b0_in), (b1_sb, b1_in)):
                dma(out=sb[:], in_=di.ap())
            ident = consts.tile([128, 128], bf16)
            make_identity(nc, ident[:])
            # switch the Q7 library early and warm it with a dummy gather:
            # the ~35us lazy ucode page-in overlaps the prelude instead of
            # sitting on the critical tail before the real gather
            nc.gpsimd.load_library(library_config.ap_gather)
            warm_idx = consts.tile([128, 1], mybir.dt.int16)
            warm_buf = consts.tile([128, 32], f32)
            nc.vector.memset(warm_idx[:], 0)
            nc.vector.memset(warm_buf[:, 0:16], 0.0)
            nc.gpsimd.ap_gather(warm_buf[:, 16:32], warm_buf[:, 0:16],
                                warm_idx[:], channels=128, num_elems=16,
                                d=1, num_idxs=16)
            ones_col = consts.tile([128, 1], bf16)
            nc.vector.memset(ones_col[:], 1.0)
            ones_row = consts.tile([1, 128], f32)
            nc.vector.memset(ones_row[:], 1.0)
            ones_rowb = consts.tile([1, 128], bf16)
            nc.vector.memset(ones_rowb[:], 1.0)

            # ---------- window indices on device (only needs u) ----------
            # computed in ap_gather's wrapped layout [128, b*32+s]
            NWB = _BS * NW
            u16 = consts.tile([128, NWB], f32)
            tf = consts.tile([128, NWB], f32)
            dma(out=u16[:], in_=u16_in.ap())
            dma(out=tf[:], in_=tw_in.ap())
            ks = small.tile([128, NWB], f32, tag="ks")
            nc.vector.tensor_scalar_mul(ks[:], tf[:], 1.0 / (T - 1))
            # the oracle's (k*s).astype(int32) rounds-to-nearest on neuron,
            # so the carry fires at 0.5 rather than 1.0
            nc.vector.tensor_scalar(ks[:], ks[:], 0.5, None, OP.is_ge)
            nc.vector.tensor_add(ks[:], tf[:], ks[:])          # center
            lo = small.tile([128, NWB], f32, tag="lo")
            nc.vector.tensor_scalar(lo[:], ks[:], -float(_W), 0.0,
                                    OP.add, OP.max)
            hi = small.tile([128, NWB], f32, tag="hi")
            nc.vector.tensor_scalar(hi[:], ks[:], float(_W), float(T),
                                    OP.add, OP.min)
            cnt = small.tile([128, NWB], f32, tag="cnt")
            nc.vector.tensor_sub(cnt[:], hi[:], lo[:])
            pr = small.tile([128, NWB], f32, tag="pr")
            nc.vector.tensor_mul(pr[:], u16[:], cnt[:])
            # floor(pr) for pr in [0, 2W+1): sum of is_ge thresholds
            fr = small.tile([128, NWB], f32, tag="fr")
            nc.vector.memset(fr[:], 0.0)
            for kth in range(1, 2 * _W + 1):
                nc.vector.scalar_tensor_tensor(
                    fr[:], pr[:], float(kth), fr[:], OP.is_ge, OP.add)
            nc.vector.tensor_add(lo[:], lo[:], fr[:])          # i2 (pre-min)
            nc.vector.tensor_scalar_add(hi[:], hi[:], -1.0)
            nc.vector.tensor_tensor(lo[:], lo[:], hi[:], op=OP.min)
            # offset row b's indices by b*T: one gather covers all rows
            for b in range(1, _BS):
                nc.vector.tensor_scalar_add(
                    lo[:, b * NW:(b + 1) * NW], lo[:, b * NW:(b + 1) * NW],
                    float(b * T))
            idx16 = consts.tile([128, NWB], mybir.dt.int16)
            nc.vector.tensor_copy(idx16[:], lo[:])

            # ---------- LSTM ----------
            def xg_precompute(ps_big, wiT_sb, K, srcs_f, srcs_b, bias_sb,
                              XGs):
                # XG layout: [128, (t, gate, s)] with gf = GF per step,
                # chunked into NFC tiles so the recurrence can start as
                # soon as chunk 0 is written
                for ch in range(NFC):
                    XG3 = XGs[ch][:].rearrange("p (t gf) -> p t gf", gf=GF)
                    for g in range(4):
                        ps = ps_big.tile([128, FC], f32, tag="xgps")
                        nc.tensor.matmul(
                            out=ps[0:_H, :],
                            lhsT=wiT_sb[0:K, g * 64:(g + 1) * 64],
                            rhs=srcs_f[ch],
                            start=True, stop=True)
                        nc.tensor.matmul(
                            out=ps[_H:128, :],
                            lhsT=wiT_sb[0:K, 256 + g * 64:256 + (g + 1) * 64],
                            rhs=srcs_b[ch],
                            start=True, stop=True)
                        dst = XG3[:, :, g * S:(g + 1) * S]
                        nc.scalar.activation(
                            dst,
                            ps[:].rearrange("p (t s) -> p t s", s=S),
                            AF.Identity, bias=bias_sb[:, g:g + 1])

            def recurrence(ps_gate, wh_sb, XGs, Ht, Hr=None, lyr=0):
                # Cell state via tensor_tensor_scan: we track ct = c/2, so
                #   ct_t = sf * ct_{t-1} + m1',  m1' = (u-0.5)*si
                #        (= [sf*c + si*tanh(g)] / 2 since tanh(g) = 2u-1)
                #   tanh(c) = Tanh(2*ct)  (ACT scale=2)
                # The scan's free layout interleaves a reset slot (d0=0,
                # d1=ct_prev) and an update slot (d0=sf, d1=m1') per stream.
                # sigma writes all gates strided into sgx (odd cols); even
                # cols stay 0 so sgx[8:16] is [0 sf 0 sf ...] = the scan d0.
                # R tiles: scan(t) writes [e0 n0 e1 n1 ...] into R[t%2][0:8]
                # (n_s = new ct at col 2s+1); m1'(t+1) then lands at cols
                # {2,4,6,8} of the same tile, so R[t%2][1:9] is the next d1.
                Hsts = [state.tile([128, S], bf16, tag=f"hst{lyr}{i}",
                                   name=f"hst{lyr}{i}") for i in range(3)]
                Rs = [state.tile([128, 2 * S + 2], f32, tag=f"r{lyr}{i}",
                                 name=f"r{lyr}{i}") for i in range(2)]
                Tc = [state.tile([128, S], bf16, tag=f"tc{lyr}{i}",
                                 name=f"tc{lyr}{i}") for i in range(2)]
                Sgx = [state.tile([128, 2 * GF], f32, tag=f"sg{lyr}{i}",
                                  name=f"sg{lyr}{i}") for i in range(2)]
                for i in range(3):
                    nc.vector.memset(Hsts[i][:], 0.0)
                for i in range(2):
                    nc.vector.memset(Rs[i][:], 0.0)
                    nc.vector.memset(Sgx[i][:], 0.0)

                def sview(sgx, g):
                    # [128, S, 1] view of gate g's columns {2*(g*S+s)+1}
                    v = sgx[:].rearrange("p (c z) -> p c z", z=2)
                    return v[:, g * S:(g + 1) * S, 1:2]

                def emit_copies(t, Hst):
                    rt = T - 1 - t
                    nc.gpsimd.tensor_copy(Ht[0:_H, t * S:(t + 1) * S],
                                          Hst[0:_H, :])
                    nc.gpsimd.tensor_copy(Ht[_H:128, rt * S:(rt + 1) * S],
                                          Hst[_H:128, :])
                    if Hr is not None:
                        nc.gpsimd.tensor_copy(Hr[0:_H, rt * S:(rt + 1) * S],
                                              Hst[0:_H, :])
                        nc.gpsimd.tensor_copy(Hr[_H:128, t * S:(t + 1) * S],
                                              Hst[_H:128, :])

                for t in range(T):
                    Hprev = Hsts[(t + 2) % 3]
                    Hst = Hsts[t % 3]
                    Ra = Rs[t % 2]
                    Rb = Rs[(t + 1) % 2]
                    tc = Tc[t % 2]
                    sgx = Sgx[t % 2]
                    gb = ps_gate.tile([128, GF], f32, tag="gates")
                    xgsl = XGs[t // TCH][:, (t % TCH) * GF:
                                         (t % TCH + 1) * GF]
                    # inject xg via identity matmul (clears has_written)
                    nc.tensor.matmul(out=gb[:], lhsT=ident[:],
                                     rhs=xgsl,
                                     start=True, stop=False,
                                     skip_group_check=True)
                    for g in (1, 2, 3, 0):
                        nc.tensor.matmul(
                            out=gb[:, g * S:(g + 1) * S],
                            lhsT=wh_sb[:, g * 128:(g + 1) * 128],
                            rhs=Hprev[:],
                            start=False, stop=(g == 0),
                            skip_group_check=True)
                    # sigmoid covers all gates (g-gate weights x2 on host,
                    # so its col holds u = sigmoid(2g)); split so the
                    # (i,f,g) part fires before the o-gate matmul lands;
                    # output is strided into odd cols of sgx
                    gb3 = gb[:, S:GF].rearrange("p (c z) -> p c z", z=1)
                    sgv = sgx[:].rearrange("p (c z) -> p c z", z=2)
                    nc.scalar.activation(sgv[:, S:GF, 1:2], gb3, AF.Sigmoid)
                    gb3o = gb[:, 0:S].rearrange("p (c z) -> p c z", z=1)
                    nc.scalar.activation(sgv[:, 0:S, 1:2], gb3o, AF.Sigmoid)
                    # m1' = (u - 0.5) * si into Rb cols {2,4,6,8}
                    m1o = Rb[:, 2:2 * S + 2].rearrange(
                        "p (s z) -> p s z", z=2)[:, :, 0:1]
                    nc.vector.scalar_tensor_tensor(
                        m1o, sview(sgx, 3), 0.5, sview(sgx, 1),
                        OP.subtract, OP.mult)
                    # ct scan: d0 = [0 sf 0 sf ...], d1 = [ct_prev m1' ...]
                    nc.vector.tensor_tensor_scan(
                        Ra[:, 0:2 * S], sgx[:, 4 * S:6 * S],
                        Rb[:, 1:2 * S + 1], 0.0, OP.mult, OP.add)
                    # tanh(c) = Tanh(2*ct) from odd cols of Ra
                    tanh_in = Ra[:, 1:2 * S + 1].rearrange(
                        "p (s z) -> p s z", z=2)[:, :, 0:1]
                    tc3 = tc[:].rearrange("p (s z) -> p s z", z=1)
                    nc.scalar.activation(tc3, tanh_in, AF.Tanh, scale=2.0)
                    hst3 = Hst[:].rearrange("p (s z) -> p s z", z=1)
                    nc.vector.tensor_tensor(hst3, sview(sgx, 0), tc3,
                                            op=OP.mult)
                    emit_copies(t, Hst)

            H0t = bigbuf.tile([128, TS], bf16, tag="h0t")
            H1t = bigbuf.tile([128, TS], bf16, tag="h1t")
            with (
                tc.tile_pool(name="ps_big", bufs=2, space="PSUM") as ps_big,
                tc.tile_pool(name="ps_gate", bufs=3, space="PSUM") as ps_gate,
            ):
                XGs = [bigbuf.tile([128, TCH * GF], bf16, tag=f"xg{ch}",
                                   name=f"xg{ch}")
                       for ch in range(NFC)]
                H0r = bigbuf.tile([128, TS], bf16, tag="h0r")
                srcs_f = [x_sb[:, ch * FC:(ch + 1) * FC] for ch in range(NFC)]
                srcs_b = [xr_sb[:, ch * FC:(ch + 1) * FC] for ch in range(NFC)]
                xg_precompute(ps_big, wi0_sb, _D, srcs_f, srcs_b, b0_sb, XGs)
                recurrence(ps_gate, wh0_sb, XGs, H0t, H0r, lyr=0)

                # layer 2: fwd reads H0t, bwd reads the reversed copy H0r
                # (negative-stride APs are rejected by the BIR verifier)
                srcs_f = [H0t[:, ch * FC:(ch + 1) * FC] for ch in range(NFC)]
                srcs_b = [H0r[:, ch * FC:(ch + 1) * FC] for ch in range(NFC)]
                xg_precompute(ps_big, wi1_sb, _E, srcs_f, srcs_b, b1_sb, XGs)
                recurrence(ps_gate, wh1_sb, XGs, H1t, lyr=1)

            # ---------- normalize + pack An (z1) / Z2 ----------
            An = bigbuf.tile([128, NLOC], bf16, tag="an")
            Z2f = bigbuf.tile([128, NLOC], f32, tag="z2")
            Bn = bigbuf.tile([128, NLOC], bf16, tag="bn")
            Bgf = bigbuf.tile([128, NLOC], f32, tag="bgf")
            H13 = H1t[:].rearrange("p (t s) -> p t s", s=S)
            n2 = nrm.tile([1, TS], f32, tag="n2")
            sq = bigbuf.tile([128, FC], bf16, tag="sq")
            with (
                tc.tile_pool(name="ps_nrm", bufs=4, space="PSUM") as ps_nrm,
                tc.tile_pool(name="ps_row", bufs=2, space="PSUM") as ps_row,
            ):
                for ch in range(NFC):
                    nc.vector.tensor_mul(sq[:],
                                         H1t[:, ch * FC:(ch + 1) * FC],
                                         H1t[:, ch * FC:(ch + 1) * FC])
                    psn = ps_row.tile([1, FC], f32, tag="psn")
                    nc.tensor.matmul(out=psn[:], lhsT=ones_col[:],
                                     rhs=sq[:], start=True, stop=True)
                    nc.vector.tensor_copy(n2[:, ch * FC:(ch + 1) * FC],
                                          psn[:])
                nc.vector.tensor_scalar_max(n2[:], n2[:], 1e-24)
                lnb = nrm.tile([1, TS], f32, tag="lnb")
                nc.scalar.activation(lnb[:], n2[:], AF.Ln)
                rin = nrm.tile([1, TS], bf16, tag="rin")
                nc.scalar.activation(rin[:], lnb[:], AF.Exp, scale=-0.5)
                # pack Z2 first: the gather + chunked AllGather start as
                # early as possible; An is packed during the collectives
                def pack(streams, ch, psb):
                    ps3 = psb[:].rearrange("p (t s) -> p t s", s=S)
                    h3 = H13[:, ch * TCH:(ch + 1) * TCH, :]
                    for s in streams:
                        if s < _BS:
                            dst = An[:, s * T + ch * TCH:
                                     s * T + (ch + 1) * TCH]
                        else:
                            b = s - _BS
                            dst = Z2f[:, b * T + ch * TCH:
                                      b * T + (ch + 1) * TCH]
                        nc.vector.tensor_tensor(
                            dst,
                            h3[:, :, s:s + 1].rearrange("p t o -> p (t o)"),
                            ps3[:, :, s:s + 1].rearrange("p t o -> p (t o)"),
                            op=OP.mult)

                def mkpsb(ch):
                    psb = ps_nrm.tile([128, FC], f32, tag="nps")
                    nc.tensor.matmul(out=psb[:], lhsT=ones_rowb[:],
                                     rhs=rin[:, ch * FC:(ch + 1) * FC],
                                     start=True, stop=True)
                    return psb
                psbs = []
                for ch in range(NFC):
                    psb = mkpsb(ch)
                    psbs.append(psb)
                    pack(range(_BS, S), ch, psb)

                # ---------- gather z2 -> Bn + AllGather ----------
                nc.gpsimd.ap_gather(
                    Bgf[:], Z2f[:], idx16[:],
                    channels=128, num_elems=NLOC, d=1, num_idxs=NLOC)
                nc.vector.tensor_copy(Bn[:], Bgf[:])
                dma(out=ag_in_d.ap(), in_=Bn[:])
                nc.gpsimd.collective_compute(
                    "AllGather", OP.bypass,
                    replica_groups=[list(range(_NCORES))],
                    ins=[ag_in_d.ap().opt()],
                    outs=[ag_out_d.ap().opt()])

                # pack An while the collectives run
                for ch in range(NFC):
                    pack(range(_BS), ch, psbs[ch])

            Ball = bigbuf.tile([128, NGLOB], bf16, tag="ball")
            for jj in range(_NCORES):
                dma(out=Ball[:, jj * NLOC:(jj + 1) * NLOC],
                    in_=ag_out_d.ap()[jj * 128:(jj + 1) * 128, :])

            # ---------- output partials ----------
            outp = consts.tile([128, 4], f32)
            nc.vector.memset(outp[:], 0.0)

            with (
                tc.tile_pool(name="ps_d", bufs=1, space="PSUM") as ps_d,
                tc.tile_pool(name="ps_s", bufs=2, space="PSUM") as ps_s,
                tc.tile_pool(name="ps_c", bufs=1, space="PSUM") as ps_c,
            ):
                # diag: sum_i <An_i, Bn_i>
                dg = bigbuf.tile([128, NLOC], bf16, tag="dg")
                nc.vector.tensor_mul(dg[:], An[:], Bn[:])
                ndc = (NLOC + 511) // 512
                psd = ps_d.tile([1, 512], f32, tag="psd")
                for ch in range(ndc):
                    nc.tensor.matmul(out=psd[:], lhsT=ones_col[:],
                                     rhs=dg[:, ch * 512:(ch + 1) * 512],
                                     start=(ch == 0), stop=(ch == ndc - 1))
                nc.vector.reduce_sum(outp[0:1, 2:3], psd[:],
                                     axis=mybir.AxisListType.X)

                # one-pass row+col logsumexp sweep ([128,1024] exp
                # blocks; row sums on DVE, col sums via ones-matmuls)
                CP = 2 * CC
                NCP = NGLOB // CP
                rows = bigbuf.tile([128, NRC * NCP], f32, tag="rows")
                csum = nrm.tile([1, NGLOB], f32, tag="csum")
                for cb in range(NCP):
                    csp0 = ps_c.tile([1, CC], f32, tag="csp0")
                    csp1 = ps_c.tile([1, CC], f32, tag="csp1")
                    for rc in range(NRC):
                        ps = ps_s.tile([128, CP], f32, tag="sps")
                        nc.tensor.matmul(
                            out=ps[:, 0:CC],
                            lhsT=An[:, rc * 128:(rc + 1) * 128],
                            rhs=Ball[:, cb * CP:cb * CP + CC],
                            start=True, stop=True)
                        nc.tensor.matmul(
                            out=ps[:, CC:CP],
                            lhsT=An[:, rc * 128:(rc + 1) * 128],
                            rhs=Ball[:, cb * CP + CC:(cb + 1) * CP],
                            start=True, stop=True)
                        eb = sgp.tile([128, CP], bf16, tag="eb")
                        nc.scalar.activation(
                            eb[:], ps[:], AF.Exp, scale=1.0 / _TEMP)
                        nc.vector.reduce_sum(
                            rows[:, rc * NCP + cb:rc * NCP + cb + 1],
                            eb[:], axis=mybir.AxisListType.X)
                        nc.tensor.matmul(
                            out=csp0[:], lhsT=ones_col[:], rhs=eb[:, 0:CC],
                            start=(rc == 0), stop=(rc == NRC - 1))
                        nc.tensor.matmul(
                            out=csp1[:], lhsT=ones_col[:], rhs=eb[:, CC:CP],
                            start=(rc == 0), stop=(rc == NRC - 1))
                    nc.vector.tensor_copy(csum[:, cb * CP:cb * CP + CC],
                                          csp0[:])
                    nc.vector.tensor_copy(
                        csum[:, cb * CP + CC:(cb + 1) * CP], csp1[:])

                tot = small.tile([128, NRC], f32, tag="tot")
                for rc in range(NRC):
                    nc.vector.reduce_sum(tot[:, rc:rc + 1],
                                         rows[:, rc * NCP:(rc + 1) * NCP],
                                         axis=mybir.AxisListType.X)
                lse = small.tile([128, NRC], f32, tag="lse")
                nc.scalar.activation(lse[:], tot[:], AF.Ln)
                nc.vector.reduce_sum(outp[:, 0:1], lse[:],
                                     axis=mybir.AxisListType.X)

            dma(out=out_d.ap(), in_=outp[:])
            dma(out=cs_d.ap(), in_=csum[:])
            if dbg:
                anb32 = bigbuf.tile([128, NLOC], f32, tag="anb32")
                for i, src in enumerate((An, Bn, Z2f)):
                    nc.vector.tensor_copy(anb32[:], src[:])
                    dma(out=anb_d.ap()[:, i * NLOC:(i + 1) * NLOC],
                        in_=anb32[:])
                nc.vector.tensor_copy(anb32[:], H1t[:, 0:NLOC])
                dma(out=anb_d.ap()[:, 3 * NLOC:4 * NLOC], in_=anb32[:])

    nc.compile()
    return nc


def _host_prep(x1, x2, u, wih0, whh0, bih0, bhh0, wih1, whh1, bih1, bhh1, T):
    """Build per-core input maps (all host work is pure data layout)."""
    import ml_dtypes
    bf16 = ml_dtypes.bfloat16

    # g-gate (kernel position _GTANH) weights/bias are pre-scaled by 2 so
    # a single sigmoid gives u = sigmoid(2g), tanh(g) = 2u - 1
    def gate_stack_T(w):
        # w: [2, 256, K] -> [K, 512]; cols = dir*256 + gperm_gate*64 + j
        K = w.shape[2]
        out = np.empty((K, 512), np.float32)
        for d in range(2):
            for gi, g in enumerate(_GPERM):
                sc = 2.0 if gi == _GTANH else 1.0
                out[:, d * 256 + gi * 64:d * 256 + (gi + 1) * 64] = \
                    sc * w[d, g * 64:(g + 1) * 64, :].T
        return out.astype(bf16)

    def blockdiag(w):
        # w: [2, 256, H] -> [128, 512]; per new-gate [128,128] block-diag
        out = np.zeros((128, 512), np.float32)
        for gi, g in enumerate(_GPERM):
            sc = 2.0 if gi == _GTANH else 1.0
            out[0:_H, gi * 128:gi * 128 + 64] = \
                sc * w[0, g * 64:(g + 1) * 64, :].T
            out[_H:128, gi * 128 + 64:(gi + 1) * 128] = \
                sc * w[1, g * 64:(g + 1) * 64, :].T
        return out.astype(bf16)

    def biases(bi, bh):
        b = bi + bh  # [2, 256]
        out = np.empty((128, 4), np.float32)
        for gi, g in enumerate(_GPERM):
            sc = 2.0 if gi == _GTANH else 1.0
            out[0:_H, gi] = sc * b[0, g * 64:(g + 1) * 64]
            out[_H:128, gi] = sc * b[1, g * 64:(g + 1) * 64]
        return out

    # t-ramp in ap_gather's wrapped layout: tw[p, b*NW+s] = s*16 + p%16
    NW = T // 16
    ramp = (np.arange(NW)[:, None] * 16 +
            np.arange(16)[None, :]).astype(np.float32)   # [NW, 16]
    twt = np.tile(ramp.T, (8, _BS))                      # [128, BS*NW]

    def wrap_u(uc):
        # uc: [BS, T] -> [128, BS*NW] wrapped + replicated per 16-part group
        cols = [np.tile(uc[b].reshape(NW, 16).T, (8, 1)) for b in range(_BS)]
        return np.ascontiguousarray(np.concatenate(cols, axis=1))

    shared = {
        "wi0T": np.ascontiguousarray(gate_stack_T(wih0)),
        "wi1T": np.ascontiguousarray(gate_stack_T(wih1)),
        "wh0bd": blockdiag(whh0),
        "wh1bd": blockdiag(whh1),
        "bias0": biases(bih0, bhh0),
        "bias1": biases(bih1, bhh1),
        "tw": np.ascontiguousarray(twt),
    }
    in_maps = []
    for k in range(_NCORES):
        rows = [x1[2 * k, :T], x1[2 * k + 1, :T], x2[2 * k, :T],
                x2[2 * k + 1, :T]]
        arr = np.stack(rows, axis=2)            # [T, D, S]
        xc = np.ascontiguousarray(
            arr.transpose(1, 0, 2).reshape(_D, T * _S)).astype(bf16)
        xr = np.ascontiguousarray(
            arr[::-1].transpose(1, 0, 2).reshape(_D, T * _S)).astype(bf16)
        m = dict(shared)
        m["x_cat"] = xc
        m["x_rev"] = xr
        m["u16"] = wrap_u(u[2 * k:2 * k + 2, :T])
        in_maps.append(m)
    return in_maps


def _run(inputs, T=_T, trace=False, dbg=False):
    from concourse import bass_utils
    key = (T, dbg)
    if key not in _cache:
        _cache[key] = _build(T, dbg)
    nc = _cache[key]
    in_maps = _host_prep(T=T, **inputs)
    res = bass_utils.run_bass_kernel_spmd(
        nc, in_maps, core_ids=list(range(_NCORES)), trace=trace)
    N = _NCORES * _BS * T
    R = sum(float(r["outp"][:, 0].sum()) for r in res.results)
    Draw = sum(float(r["outp"][0, 2]) for r in res.results)
    colsum = np.zeros(N, np.float64)
    for r in res.results:
        colsum += np.asarray(r["colsum"][0], np.float64)
    C = float(np.log(colsum).sum())
    Dg = Draw / _TEMP
    loss = -((Dg - R) / N + (Dg - C) / N)
    return np.float32(loss), res


def kernel(**inputs):
    loss, _ = _run(inputs)
    return np.asarray(loss, dtype=np.float32)



# revision 4
# speedup vs baseline: 1.0536x; 1.0042x over previous
"""Trainium2 Bass kernel for nn_MIPS_74904229642848 (v3).

Pipeline (8 NeuronCores, SPMD, batch-sharded 2 rows/core, S=4 streams/core):
  1. 2-layer bidirectional LSTM, all-bf16 matmuls. Per step: one identity
     matmul injects the precomputed input-gate terms into PSUM (chunked so
     each recurrence starts after its first xg chunk), four bf16 block-diag
     recurrence matmuls accumulate on top. The g-gate weights/bias are
     pre-scaled by 2 on the host so ONE sigmoid over all four gates covers
     the tanh as well (tanh(g) = 2*sigmoid(2g)-1); the cell update is two
     fused scalar_tensor_tensor ops on DVE (m1' = (u-.5)*si;
     c = 2*m1' + m2) with m2 = sf*c on GpSimd, c in SBUF, tanh(c) on ACT,
     h = so*tanh(c) on DVE into a triple-buffered Hst.
  2. L2 normalization via ln/exp rsqrt (no Newton), fused scale+pack.
  3. Windowed index sampling of z2 via indirect DMA (bf16).
  4. AllGather of the B-side embeddings only (bf16).
  5. One-pass logits sweep: bf16 sim matmul blocks, exp (ACT, accum_out
     gives row sums), ones-matmul accumulates column sums in PSUM across
     row blocks. Row-lse finished on device; per-core column-sum partials
     shipped to the host, which does the final ln+sum combine.
"""

import numpy as np

_D, _E, _H, _B, _W = 64, 128, 64, 16, 3
_T = 512
_TEMP = 0.05
_NCORES = 8
_BS = _B // _NCORES          # batch rows per core
_S = 2 * _BS                 # streams per core: (x1,b0),(x1,b1),(x2,b0),(x2,b1)
_GF = 4 * _S                 # gate-block width per step

# torch gate order i,f,g,o -> kernel order o,i,f,g (tanh block last)
_GPERM = [3, 0, 1, 2]
_GTANH = 3                   # index of the g gate in kernel order

_cache = {}


def _build(T, dbg=False):
    import concourse.bass as bass
    import concourse.mybir as mybir
    import concourse.tile as tile
    from concourse import bacc, library_config
    from concourse.masks import make_identity

    f32 = mybir.dt.float32
    bf16 = mybir.dt.bfloat16
    i32 = mybir.dt.int32
    AF = mybir.ActivationFunctionType
    OP = mybir.AluOpType

    S = _S
    GF = _GF
    TS = T * S
    NLOC = _BS * T
    NGLOB = _NCORES * NLOC
    FC = min(512, TS)            # xg free chunk
    NFC = TS // FC
    TCH = FC // S                # timesteps per xg chunk
    CC = min(512, NGLOB)         # logits col chunk
    NCC = NGLOB // CC
    NRC = (NLOC + 127) // 128    # logits row chunks (M=128)
    NTC = T // 128               # transpose chunks per stream
    NUC = T // 128

    nc = bacc.Bacc("TRN2", target_bir_lowering=False, debug=False,
                   num_devices=_NCORES)

    # ---- I/O ----
    x_in = nc.dram_tensor("x_cat", [_D, TS], bf16, kind="ExternalInput")
    xr_in = nc.dram_tensor("x_rev", [_D, TS], bf16, kind="ExternalInput")
    u2_in = nc.dram_tensor("u2", [1, _BS * T], f32, kind="ExternalInput")
    tw_in = nc.dram_tensor("tw2", [1, _BS * T], f32, kind="ExternalInput")
    wi0_in = nc.dram_tensor("wi0T", [_D, 512], bf16, kind="ExternalInput")
    wi1_in = nc.dram_tensor("wi1T", [_E, 512], bf16, kind="ExternalInput")
    wh0_in = nc.dram_tensor("wh0bd", [_E, 512], bf16, kind="ExternalInput")
    wh1_in = nc.dram_tensor("wh1bd", [_E, 512], bf16, kind="ExternalInput")
    b0_in = nc.dram_tensor("bias0", [_E, 4], f32, kind="ExternalInput")
    b1_in = nc.dram_tensor("bias1", [_E, 4], f32, kind="ExternalInput")
    out_d = nc.dram_tensor("outp", [128, 4], f32, kind="ExternalOutput")
    cs_d = nc.dram_tensor("colsum", [1, NGLOB], f32, kind="ExternalOutput")
    if dbg:
        anb_d = nc.dram_tensor("anb", [128, 4 * NLOC], f32,
                               kind="ExternalOutput")

    # DRAM scratch for the AllGather
    ag_in_d = nc.dram_tensor("ag_in", [128, NLOC], bf16)
    ag_out_d = nc.dram_tensor("ag_out", [_NCORES * 128, NLOC], bf16,
                              addr_space="Shared")

    with tile.TileContext(nc) as tc:
        with (
            tc.tile_pool(name="consts", bufs=1) as consts,
            tc.tile_pool(name="bigbuf", bufs=1) as bigbuf,
            tc.tile_pool(name="state", bufs=1) as state,
            tc.tile_pool(name="small", bufs=3) as small,
            tc.tile_pool(name="nrm", bufs=1) as nrm,
            tc.tile_pool(name="sg", bufs=4) as sgp,
        ):
            dma = nc.sync.dma_start

            # ---------- load constants / inputs ----------
            x_sb = bigbuf.tile([_D, TS], bf16, tag="x")
            xr_sb = bigbuf.tile([_D, TS], bf16, tag="xr")
            dma(out=x_sb[:], in_=x_in.ap())
            dma(out=xr_sb[:], in_=xr_in.ap())
            wi0_sb = consts.tile([_D, 512], bf16)
            wi1_sb = consts.tile([_E, 512], bf16)
            wh0_sb = consts.tile([_E, 512], bf16)
            wh1_sb = consts.tile([_E, 512], bf16)
            b0_sb = consts.tile([_E, 4], f32)
            b1_sb = consts.tile([_E, 4], f32)
            for sb, di in ((wi0_sb, wi0_in), (wi1_sb, wi1_in),
                           (wh0_sb, wh0_in), (wh1_sb, wh1_in),
                           (b0_sb, b0_in), (b1_sb, b1_in)):
                dma(out=sb[:], in_=di.ap())
            ident = consts.tile([128, 128], bf16)
            make_identity(nc, ident[:])
            ones_col = consts.tile([128, 1], bf16)
            nc.vector.memset(ones_col[:], 1.0)
            ones_row = consts.tile([1, 128], f32)
            nc.vector.memset(ones_row[:], 1.0)
            ones_rowb = consts.tile([1, 128], bf16)
            nc.vector.memset(ones_rowb[:], 1.0)

            # ---------- window offsets on device (only needs u) ----------
            # o[b,t] = i2[b,t] - t in [-3, 4]; the z2 sampling is then 8
            # one-hot masked shift-accumulates (no gpsimd gather needed)
            ob = consts.tile([1, _BS * T], bf16)
            with tc.tile_pool(name="idxp", bufs=1) as idxp:
                u2 = idxp.tile([1, _BS * T], f32, name="u2t")
                tf = idxp.tile([1, _BS * T], f32, name="tft")
                dma(out=u2[:], in_=u2_in.ap())
                dma(out=tf[:], in_=tw_in.ap())
                ks = idxp.tile([1, _BS * T], f32, name="kst")
                nc.vector.tensor_scalar_mul(ks[:], tf[:], 1.0 / (T - 1))
                # the oracle's (k*s).astype(int32) rounds-to-nearest on
                # neuron, so the carry fires at 0.5 rather than 1.0
                nc.vector.tensor_scalar(ks[:], ks[:], 0.5, None, OP.is_ge)
                nc.vector.tensor_add(ks[:], tf[:], ks[:])      # center
                lo = idxp.tile([1, _BS * T], f32, name="lot")
                nc.vector.tensor_scalar(lo[:], ks[:], -float(_W), 0.0,
                                        OP.add, OP.max)
                hi = idxp.tile([1, _BS * T], f32, name="hit")
                nc.vector.tensor_scalar(hi[:], ks[:], float(_W), float(T),
                                        OP.add, OP.min)
                cnt = idxp.tile([1, _BS * T], f32, name="cntt")
                nc.vector.tensor_sub(cnt[:], hi[:], lo[:])
                nc.vector.tensor_mul(cnt[:], u2[:], cnt[:])    # pr
                fr = idxp.tile([1, _BS * T], f32, name="frt")
                nc.vector.memset(fr[:], 0.0)
                for kth in range(1, 2 * _W + 1):
                    nc.vector.scalar_tensor_tensor(
                        fr[:], cnt[:], float(kth), fr[:], OP.is_ge, OP.add)
                nc.vector.tensor_add(lo[:], lo[:], fr[:])      # i2 (pre-min)
                nc.vector.tensor_scalar_add(hi[:], hi[:], -1.0)
                nc.vector.tensor_tensor(lo[:], lo[:], hi[:], op=OP.min)
                nc.vector.tensor_sub(lo[:], lo[:], tf[:])      # o = i2 - t
                nc.vector.tensor_copy(ob[:], lo[:])
            # broadcast o across partitions (1-contraction matmuls) and
            # build the 8 one-hot masks; overlaps the LSTM epilogue
            ones_rowb0 = consts.tile([1, 128], bf16)
            nc.vector.memset(ones_rowb0[:], 1.0)
            # mask storage is aliased into the XG tiles and H0r, which
            # are dead once the layer-2 recurrence has consumed them

            # ---------- LSTM ----------
            def xg_precompute(ps_big, wiT_sb, K, srcs_f, srcs_b, bias_sb,
                              XGs):
                # XG layout: [128, (t, gate, s)] with gf = GF per step,
                # chunked into NFC tiles so the recurrence can start as
                # soon as chunk 0 is written
                for ch in range(NFC):
                    XG3 = XGs[ch][:].rearrange("p (t gf) -> p t gf", gf=GF)
                    for g in range(4):
                        ps = ps_big.tile([128, FC], f32, tag="xgps")
                        nc.tensor.matmul(
                            out=ps[0:_H, :],
                            lhsT=wiT_sb[0:K, g * 64:(g + 1) * 64],
                            rhs=srcs_f[ch],
                            start=True, stop=True)
                        nc.tensor.matmul(
                            out=ps[_H:128, :],
                            lhsT=wiT_sb[0:K, 256 + g * 64:256 + (g + 1) * 64],
                            rhs=srcs_b[ch],
                            start=True, stop=True)
                        dst = XG3[:, :, g * S:(g + 1) * S]
                        nc.scalar.activation(
                            dst,
                            ps[:].rearrange("p (t s) -> p t s", s=S),
                            AF.Identity, bias=bias_sb[:, g:g + 1])

            def recurrence(ps_gate, wh_sb, XGs, Ht, Hr=None, lyr=0):
                # Cell state via tensor_tensor_scan: we track ct = c/2, so
                #   ct_t = sf * ct_{t-1} + m1',  m1' = (u-0.5)*si
                #        (= [sf*c + si*tanh(g)] / 2 since tanh(g) = 2u-1)
                #   tanh(c) = Tanh(2*ct)  (ACT scale=2)
                # The scan's free layout interleaves a reset slot (d0=0,
                # d1=ct_prev) and an update slot (d0=sf, d1=m1') per stream.
                # sigma writes all gates strided into sgx (odd cols); even
                # cols stay 0 so sgx[8:16] is [0 sf 0 sf ...] = the scan d0.
                # R tiles: scan(t) writes [e0 n0 e1 n1 ...] into R[t%2][0:8]
                # (n_s = new ct at col 2s+1); m1'(t+1) then lands at cols
                # {2,4,6,8} of the same tile, so R[t%2][1:9] is the next d1.
                Hsts = [state.tile([128, S], bf16, tag=f"hst{lyr}{i}",
                                   name=f"hst{lyr}{i}") for i in range(3)]
                Rs = [state.tile([128, 2 * S + 2], f32, tag=f"r{lyr}{i}",
                                 name=f"r{lyr}{i}") for i in range(2)]
                Tc = [state.tile([128, S], bf16, tag=f"tc{lyr}{i}",
                                 name=f"tc{lyr}{i}") for i in range(2)]
                Sgx = [state.tile([128, 2 * GF], f32, tag=f"sg{lyr}{i}",
                                  name=f"sg{lyr}{i}") for i in range(2)]
                for i in range(3):
                    nc.vector.memset(Hsts[i][:], 0.0)
                for i in range(2):
                    nc.vector.memset(Rs[i][:], 0.0)
                    nc.vector.memset(Sgx[i][:], 0.0)

                def sview(sgx, g):
                    # [128, S, 1] view of gate g's columns {2*(g*S+s)+1}
                    v = sgx[:].rearrange("p (c z) -> p c z", z=2)
                    return v[:, g * S:(g + 1) * S, 1:2]

                def emit_copies(t, Hst):
                    rt = T - 1 - t
                    nc.gpsimd.tensor_copy(Ht[0:_H, t * S:(t + 1) * S],
                                          Hst[0:_H, :])
                    nc.gpsimd.tensor_copy(Ht[_H:128, rt * S:(rt + 1) * S],
                                          Hst[_H:128, :])
                    if Hr is not None:
                        nc.gpsimd.tensor_copy(Hr[0:_H, rt * S:(rt + 1) * S],
                                              Hst[0:_H, :])
                        nc.gpsimd.tensor_copy(Hr[_H:128, t * S:(t + 1) * S],
                                              Hst[_H:128, :])

                for t in range(T):
                    Hprev = Hsts[(t + 2) % 3]
                    Hst = Hsts[t % 3]
                    Ra = Rs[t % 2]
                    Rb = Rs[(t + 1) % 2]
                    tc = Tc[t % 2]
                    sgx = Sgx[t % 2]
                    gb = ps_gate.tile([128, GF], f32, tag="gates")
                    xgsl = XGs[t // TCH][:, (t % TCH) * GF:
                                         (t % TCH + 1) * GF]
                    # inject xg via identity matmul (clears has_written)
                    nc.tensor.matmul(out=gb[:], lhsT=ident[:],
                                     rhs=xgsl,
                                     start=True, stop=False,
                                     skip_group_check=True)
                    for g in (1, 2, 3, 0):
                        nc.tensor.matmul(
                            out=gb[:, g * S:(g + 1) * S],
                            lhsT=wh_sb[:, g * 128:(g + 1) * 128],
                            rhs=Hprev[:],
                            start=False, stop=(g == 0),
                            skip_group_check=True)
                    # sigmoid covers all gates (g-gate weights x2 on host,
                    # so its col holds u = sigmoid(2g)); split so the
                    # (i,f,g) part fires before the o-gate matmul lands;
                    # output is strided into odd cols of sgx
                    gb3 = gb[:, S:GF].rearrange("p (c z) -> p c z", z=1)
                    sgv = sgx[:].rearrange("p (c z) -> p c z", z=2)
                    nc.scalar.activation(sgv[:, S:GF, 1:2], gb3, AF.Sigmoid)
                    gb3o = gb[:, 0:S].rearrange("p (c z) -> p c z", z=1)
                    nc.scalar.activation(sgv[:, 0:S, 1:2], gb3o, AF.Sigmoid)
                    # m1' = (u - 0.5) * si into Rb cols {2,4,6,8}
                    m1o = Rb[:, 2:2 * S + 2].rearrange(
                        "p (s z) -> p s z", z=2)[:, :, 0:1]
                    nc.vector.scalar_tensor_tensor(
                        m1o, sview(sgx, 3), 0.5, sview(sgx, 1),
                        OP.subtract, OP.mult)
                    # ct scan: d0 = [0 sf 0 sf ...], d1 = [ct_prev m1' ...]
                    nc.vector.tensor_tensor_scan(
                        Ra[:, 0:2 * S], sgx[:, 4 * S:6 * S],
                        Rb[:, 1:2 * S + 1], 0.0, OP.mult, OP.add)
                    # tanh(c) = Tanh(2*ct) from odd cols of Ra
                    tanh_in = Ra[:, 1:2 * S + 1].rearrange(
                        "p (s z) -> p s z", z=2)[:, :, 0:1]
                    tc3 = tc[:].rearrange("p (s z) -> p s z", z=1)
                    nc.scalar.activation(tc3, tanh_in, AF.Tanh, scale=2.0)
                    hst3 = Hst[:].rearrange("p (s z) -> p s z", z=1)
                    nc.vector.tensor_tensor(hst3, sview(sgx, 0), tc3,
                                            op=OP.mult)
                    emit_copies(t, Hst)

            H0t = bigbuf.tile([128, TS], bf16, tag="h0t")
            H1t = bigbuf.tile([128, TS], bf16, tag="h1t")
            with (
                tc.tile_pool(name="ps_big", bufs=2, space="PSUM") as ps_big,
                tc.tile_pool(name="ps_gate", bufs=3, space="PSUM") as ps_gate,
            ):
                XGs = [bigbuf.tile([128, TCH * GF], bf16, tag=f"xg{ch}",
                                   name=f"xg{ch}")
                       for ch in range(NFC)]
                H0r = bigbuf.tile([128, TS], bf16, tag="h0r")
                srcs_f = [x_sb[:, ch * FC:(ch + 1) * FC] for ch in range(NFC)]
                srcs_b = [xr_sb[:, ch * FC:(ch + 1) * FC] for ch in range(NFC)]
                xg_precompute(ps_big, wi0_sb, _D, srcs_f, srcs_b, b0_sb, XGs)
                recurrence(ps_gate, wh0_sb, XGs, H0t, H0r, lyr=0)

                # layer 2: fwd reads H0t, bwd reads the reversed copy H0r
                # (negative-stride APs are rejected by the BIR verifier)
                srcs_f = [H0t[:, ch * FC:(ch + 1) * FC] for ch in range(NFC)]
                srcs_b = [H0r[:, ch * FC:(ch + 1) * FC] for ch in range(NFC)]
                xg_precompute(ps_big, wi1_sb, _E, srcs_f, srcs_b, b1_sb, XGs)
                recurrence(ps_gate, wh1_sb, XGs, H1t, lyr=1)

            # ---------- normalize + pack An (z1) / Z2 ----------
            An = bigbuf.tile([128, NLOC], bf16, tag="an")
            Z2f = bigbuf.tile([128, NLOC + 8], bf16, tag="z2")
            nc.vector.memset(Z2f[:, 0:4], 0.0)
            nc.vector.memset(Z2f[:, NLOC + 4:NLOC + 8], 0.0)
            Bn = bigbuf.tile([128, NLOC], bf16, tag="bn")
            masks = [XGs[dd // 2][:, (dd % 2) * NLOC:(dd % 2 + 1) * NLOC]
                     for dd in range(2 * _W + 2)]
            osb = H0r[:, 0:NLOC]
            tmpb = H0r[:, NLOC:2 * NLOC]
            with tc.tile_pool(name="ps_ob", bufs=2, space="PSUM") as ps_ob:
                for b in range(_BS):
                    pso = ps_ob.tile([128, T], f32, tag="pso")
                    nc.tensor.matmul(out=pso[:], lhsT=ones_rowb0[:],
                                     rhs=ob[:, b * T:(b + 1) * T],
                                     start=True, stop=True)
                    nc.vector.tensor_copy(osb[:, b * T:(b + 1) * T], pso[:])
            for dd in range(2 * _W + 2):
                nc.vector.tensor_scalar(masks[dd], osb[:],
                                        float(dd - _W), None, OP.is_equal)
            H13 = H1t[:].rearrange("p (t s) -> p t s", s=S)
            n2 = nrm.tile([1, TS], f32, tag="n2")
            sq = bigbuf.tile([128, FC], bf16, tag="sq")
            with (
                tc.tile_pool(name="ps_nrm", bufs=4, space="PSUM") as ps_nrm,
                tc.tile_pool(name="ps_row", bufs=2, space="PSUM") as ps_row,
            ):
                for ch in range(NFC):
                    nc.vector.tensor_mul(sq[:],
                                         H1t[:, ch * FC:(ch + 1) * FC],
                                         H1t[:, ch * FC:(ch + 1) * FC])
                    psn = ps_row.tile([1, FC], f32, tag="psn")
                    nc.tensor.matmul(out=psn[:], lhsT=ones_col[:],
                                     rhs=sq[:], start=True, stop=True)
                    nc.vector.tensor_copy(n2[:, ch * FC:(ch + 1) * FC],
                                          psn[:])
                nc.vector.tensor_scalar_max(n2[:], n2[:], 1e-24)
                lnb = nrm.tile([1, TS], f32, tag="lnb")
                nc.scalar.activation(lnb[:], n2[:], AF.Ln)
                rin = nrm.tile([1, TS], bf16, tag="rin")
                nc.scalar.activation(rin[:], lnb[:], AF.Exp, scale=-0.5)
                # pack Z2 first: the gather + chunked AllGather start as
                # early as possible; An is packed during the collectives
                def pack(streams, ch, psb):
                    ps3 = psb[:].rearrange("p (t s) -> p t s", s=S)
                    h3 = H13[:, ch * TCH:(ch + 1) * TCH, :]
                    for s in streams:
                        if s < _BS:
                            dst = An[:, s * T + ch * TCH:
                                     s * T + (ch + 1) * TCH]
                        else:
                            b = s - _BS
                            dst = Z2f[:, 4 + b * T + ch * TCH:
                                      4 + b * T + (ch + 1) * TCH]
                        nc.vector.tensor_tensor(
                            dst,
                            h3[:, :, s:s + 1].rearrange("p t o -> p (t o)"),
                            ps3[:, :, s:s + 1].rearrange("p t o -> p (t o)"),
                            op=OP.mult)

                def mkpsb(ch):
                    psb = ps_nrm.tile([128, FC], f32, tag="nps")
                    nc.tensor.matmul(out=psb[:], lhsT=ones_rowb[:],
                                     rhs=rin[:, ch * FC:(ch + 1) * FC],
                                     start=True, stop=True)
                    return psb
                psbs = []
                for ch in range(NFC):
                    psb = mkpsb(ch)
                    psbs.append(psb)
                    pack(range(_BS, S), ch, psb)

                # ---------- gather z2 -> Bn: one-hot shift-accumulate ----
                # Bn[:,t] = sum_d mask_d[t] * Z2f[:, 4+t+d]; the clamped
                # index computation guarantees masked-out terms never pick
                # the padding or a neighboring row's data
                nc.vector.tensor_tensor(Bn[:], Z2f[:, 1:NLOC + 1],
                                        masks[0], op=OP.mult)
                for dd in range(1, 2 * _W + 2):
                    nc.vector.tensor_tensor(
                        tmpb, Z2f[:, 1 + dd:NLOC + 1 + dd],
                        masks[dd], op=OP.mult)
                    nc.vector.tensor_add(Bn[:], Bn[:], tmpb)
                dma(out=ag_in_d.ap(), in_=Bn[:])
                nc.gpsimd.collective_compute(
                    "AllGather", OP.bypass,
                    replica_groups=[list(range(_NCORES))],
                    ins=[ag_in_d.ap().opt()],
                    outs=[ag_out_d.ap().opt()])

                # pack An while the collectives run
                for ch in range(NFC):
                    pack(range(_BS), ch, psbs[ch])

            Ball = bigbuf.tile([128, NGLOB], bf16, tag="ball")
            for jj in range(_NCORES):
                dma(out=Ball[:, jj * NLOC:(jj + 1) * NLOC],
                    in_=ag_out_d.ap()[jj * 128:(jj + 1) * 128, :])

            # ---------- output partials ----------
            outp = consts.tile([128, 4], f32)
            nc.vector.memset(outp[:], 0.0)

            with (
                tc.tile_pool(name="ps_d", bufs=1, space="PSUM") as ps_d,
                tc.tile_pool(name="ps_s", bufs=2, space="PSUM") as ps_s,
                tc.tile_pool(name="ps_c", bufs=1, space="PSUM") as ps_c,
            ):
                # diag: sum_i <An_i, Bn_i>
                dg = bigbuf.tile([128, NLOC], bf16, tag="dg")
                nc.vector.tensor_mul(dg[:], An[:], Bn[:])
                ndc = (NLOC + 511) // 512
                psd = ps_d.tile([1, 512], f32, tag="psd")
                for ch in range(ndc):
                    nc.tensor.matmul(out=psd[:], lhsT=ones_col[:],
                                     rhs=dg[:, ch * 512:(ch + 1) * 512],
                                     start=(ch == 0), stop=(ch == ndc - 1))
                nc.vector.reduce_sum(outp[0:1, 2:3], psd[:],
                                     axis=mybir.AxisListType.X)

                # one-pass row+col logsumexp sweep ([128,1024] exp
                # blocks; row sums on DVE, col sums via ones-matmuls)
                CP = 2 * CC
                NCP = NGLOB // CP
                rows = bigbuf.tile([128, NRC * NCP], f32, tag="rows")
                csum = nrm.tile([1, NGLOB], f32, tag="csum")
                for cb in range(NCP):
                    csp0 = ps_c.tile([1, CC], f32, tag="csp0")
                    csp1 = ps_c.tile([1, CC], f32, tag="csp1")
                    for rc in range(NRC):
                        ps = ps_s.tile([128, CP], f32, tag="sps")
                        nc.tensor.matmul(
                            out=ps[:, 0:CC],
                            lhsT=An[:, rc * 128:(rc + 1) * 128],
                            rhs=Ball[:, cb * CP:cb * CP + CC],
                            start=True, stop=True)
                        nc.tensor.matmul(
                            out=ps[:, CC:CP],
                            lhsT=An[:, rc * 128:(rc + 1) * 128],
                            rhs=Ball[:, cb * CP + CC:(cb + 1) * CP],
                            start=True, stop=True)
                        eb = sgp.tile([128, CP], bf16, tag="eb")
                        nc.scalar.activation(
                            eb[:], ps[:], AF.Exp, scale=1.0 / _TEMP)
                        nc.vector.reduce_sum(
                            rows[:, rc * NCP + cb:rc * NCP + cb + 1],
                            eb[:], axis=mybir.AxisListType.X)
                        nc.tensor.matmul(
                            out=csp0[:], lhsT=ones_col[:], rhs=eb[:, 0:CC],
                            start=(rc == 0), stop=(rc == NRC - 1))
                        nc.tensor.matmul(
                            out=csp1[:], lhsT=ones_col[:], rhs=eb[:, CC:CP],
                            start=(rc == 0), stop=(rc == NRC - 1))
                    nc.vector.tensor_copy(csum[:, cb * CP:cb * CP + CC],
                                          csp0[:])
                    nc.vector.tensor_copy(
                        csum[:, cb * CP + CC:(cb + 1) * CP], csp1[:])

                tot = small.tile([128, NRC], f32, tag="tot")
                for rc in range(NRC):
                    nc.vector.reduce_sum(tot[:, rc:rc + 1],
                                         rows[:, rc * NCP:(rc + 1) * NCP],
                                         axis=mybir.AxisListType.X)
                lse = small.tile([128, NRC], f32, tag="lse")
                nc.scalar.activation(lse[:], tot[:], AF.Ln)
                nc.vector.reduce_sum(outp[:, 0:1], lse[:],
                                     axis=mybir.AxisListType.X)

            dma(out=out_d.ap(), in_=outp[:])
            dma(out=cs_d.ap(), in_=csum[:])
            if dbg:
                anb32 = bigbuf.tile([128, NLOC], f32, tag="anb32")
                for i, src in enumerate((An, Bn)):
                    nc.vector.tensor_copy(anb32[:], src[:])
                    dma(out=anb_d.ap()[:, i * NLOC:(i + 1) * NLOC],
                        in_=anb32[:])
                nc.vector.tensor_copy(anb32[:], Z2f[:, 4:NLOC + 4])
                dma(out=anb_d.ap()[:, 2 * NLOC:3 * NLOC], in_=anb32[:])
                nc.vector.tensor_copy(anb32[:], H1t[:, 0:NLOC])
                dma(out=anb_d.ap()[:, 3 * NLOC:4 * NLOC], in_=anb32[:])

    nc.compile()
    return nc


def _host_prep(x1, x2, u, wih0, whh0, bih0, bhh0, wih1, whh1, bih1, bhh1, T):
    """Build per-core input maps (all host work is pure data layout)."""
    import ml_dtypes
    bf16 = ml_dtypes.bfloat16

    # g-gate (kernel position _GTANH) weights/bias are pre-scaled by 2 so
    # a single sigmoid gives u = sigmoid(2g), tanh(g) = 2u - 1
    def gate_stack_T(w):
        # w: [2, 256, K] -> [K, 512]; cols = dir*256 + gperm_gate*64 + j
        K = w.shape[2]
        out = np.empty((K, 512), np.float32)
        for d in range(2):
            for gi, g in enumerate(_GPERM):
                sc = 2.0 if gi == _GTANH else 1.0
                out[:, d * 256 + gi * 64:d * 256 + (gi + 1) * 64] = \
                    sc * w[d, g * 64:(g + 1) * 64, :].T
        return out.astype(bf16)

    def blockdiag(w):
        # w: [2, 256, H] -> [128, 512]; per new-gate [128,128] block-diag
        out = np.zeros((128, 512), np.float32)
        for gi, g in enumerate(_GPERM):
            sc = 2.0 if gi == _GTANH else 1.0
            out[0:_H, gi * 128:gi * 128 + 64] = \
                sc * w[0, g * 64:(g + 1) * 64, :].T
            out[_H:128, gi * 128 + 64:(gi + 1) * 128] = \
                sc * w[1, g * 64:(g + 1) * 64, :].T
        return out.astype(bf16)

    def biases(bi, bh):
        b = bi + bh  # [2, 256]
        out = np.empty((128, 4), np.float32)
        for gi, g in enumerate(_GPERM):
            sc = 2.0 if gi == _GTANH else 1.0
            out[0:_H, gi] = sc * b[0, g * 64:(g + 1) * 64]
            out[_H:128, gi] = sc * b[1, g * 64:(g + 1) * 64]
        return out

    twt = np.tile(np.arange(T, dtype=np.float32), _BS)[None, :]

    shared = {
        "wi0T": np.ascontiguousarray(gate_stack_T(wih0)),
        "wi1T": np.ascontiguousarray(gate_stack_T(wih1)),
        "wh0bd": blockdiag(whh0),
        "wh1bd": blockdiag(whh1),
        "bias0": biases(bih0, bhh0),
        "bias1": biases(bih1, bhh1),
        "tw2": np.ascontiguousarray(twt),
    }
    in_maps = []
    for k in range(_NCORES):
        rows = [x1[2 * k, :T], x1[2 * k + 1, :T], x2[2 * k, :T],
                x2[2 * k + 1, :T]]
        arr = np.stack(rows, axis=2)            # [T, D, S]
        xc = np.ascontiguousarray(
            arr.transpose(1, 0, 2).reshape(_D, T * _S)).astype(bf16)
        xr = np.ascontiguousarray(
            arr[::-1].transpose(1, 0, 2).reshape(_D, T * _S)).astype(bf16)
        m = dict(shared)
        m["x_cat"] = xc
        m["x_rev"] = xr
        m["u2"] = np.ascontiguousarray(
            u[2 * k:2 * k + 2, :T].reshape(1, -1))
        in_maps.append(m)
    return in_maps


def _run(inputs, T=_T, trace=False, dbg=False):
    from concourse import bass_utils
    key = (T, dbg)
    if key not in _cache:
        _cache[key] = _build(T, dbg)
    nc = _cache[key]
    in_maps = _host_prep(T=T, **inputs)
    res = bass_utils.run_bass_kernel_spmd(
        nc, in_maps, core_ids=list(range(_NCORES)), trace=trace)
    N = _NCORES * _BS * T
    R = sum(float(r["outp"][:, 0].sum()) for r in res.results)
    Draw = sum(float(r["outp"][0, 2]) for r in res.results)
    colsum = np.zeros(N, np.float64)
    for r in res.results:
        colsum += np.asarray(r["colsum"][0], np.float64)
    C = float(np.log(colsum).sum())
    Dg = Draw / _TEMP
    loss = -((Dg - R) / N + (Dg - C) / N)
    return np.float32(loss), res


def kernel(**inputs):
    loss, _ = _run(inputs)
    return np.asarray(loss, dtype=np.float32)



# revision 5
# speedup vs baseline: 1.1062x; 1.0499x over previous
"""Trainium2 Bass kernel for nn_MIPS_74904229642848 (v3).

Pipeline (8 NeuronCores, SPMD, batch-sharded 2 rows/core, S=4 streams/core):
  1. 2-layer bidirectional LSTM, all-bf16 matmuls. Per step: one identity
     matmul injects the precomputed input-gate terms into PSUM (chunked so
     each recurrence starts after its first xg chunk), four bf16 block-diag
     recurrence matmuls accumulate on top (issue order i,f,g,o). The
     g-gate weights/bias are pre-scaled by 2 on the host so sigmoid covers
     the tanh too (tanh(g) = 2*sigmoid(2g)-1); sigma is split (i,f,g | o)
     and written strided into sgx whose even cols stay 0. The cell state
     ct = c/2 is advanced by ONE tensor_tensor_scan per step over an
     interleaved [reset|update] slot pair per stream (d0 = [0 sf ...],
     d1 = [ct_prev m1' ...], m1' = (u-.5)*si via one stt), tanh(c) =
     Tanh(2*ct) on ACT, h = so*tanh(c) on DVE into a triple-buffered Hst;
     Ht/Hr copies run on GpSimd.
  2. L2 normalization via ln/exp rsqrt, bf16 inverse-norm broadcast.
  3. Windowed sampling of z2 as 8 one-hot masked shift-accumulates on DVE
     (masks built from the on-device index offsets, aliased into the dead
     XG/H0r tiles).
  4. AllGather of the B-side embeddings only (bf16).
  5. One-pass logits sweep: bf16 sim matmul blocks, exp (ACT, accum_out
     gives row sums), ones-matmul accumulates column sums in PSUM across
     row blocks. Row-lse finished on device; per-core column-sum partials
     shipped to the host, which does the final ln+sum combine.
"""

import numpy as np

_D, _E, _H, _B, _W = 64, 128, 64, 16, 3
_T = 512
_TEMP = 0.05
_NCORES = 8
_BS = _B // _NCORES          # batch rows per core
_S = 2 * _BS                 # streams per core: (x1,b0),(x1,b1),(x2,b0),(x2,b1)
_GF = 4 * _S                 # gate-block width per step

# torch gate order i,f,g,o -> kernel order o,i,f,g (tanh block last)
_GPERM = [3, 0, 1, 2]
_GTANH = 3                   # index of the g gate in kernel order

_cache = {}


def _build(T, dbg=False):
    import concourse.bass as bass
    import concourse.mybir as mybir
    import concourse.tile as tile
    from concourse import bacc, library_config
    from concourse.masks import make_identity

    f32 = mybir.dt.float32
    bf16 = mybir.dt.bfloat16
    i32 = mybir.dt.int32
    AF = mybir.ActivationFunctionType
    OP = mybir.AluOpType

    S = _S
    GF = _GF
    TS = T * S
    NLOC = _BS * T
    NGLOB = _NCORES * NLOC
    FC = min(512, TS)            # xg free chunk
    NFC = TS // FC
    TCH = FC // S                # timesteps per xg chunk
    CC = min(512, NGLOB)         # logits col chunk
    NCC = NGLOB // CC
    NRC = (NLOC + 127) // 128    # logits row chunks (M=128)
    NTC = T // 128               # transpose chunks per stream
    NUC = T // 128

    nc = bacc.Bacc("TRN2", target_bir_lowering=False, debug=False,
                   num_devices=_NCORES)

    # ---- I/O ----
    x_in = nc.dram_tensor("x_cat", [_D, TS], bf16, kind="ExternalInput")
    xr_in = nc.dram_tensor("x_rev", [_D, TS], bf16, kind="ExternalInput")
    u2_in = nc.dram_tensor("u2", [1, _BS * T], f32, kind="ExternalInput")
    tw_in = nc.dram_tensor("tw2", [1, _BS * T], f32, kind="ExternalInput")
    wi0_in = nc.dram_tensor("wi0T", [_D, 512], bf16, kind="ExternalInput")
    wi1_in = nc.dram_tensor("wi1T", [_E, 512], bf16, kind="ExternalInput")
    wh0_in = nc.dram_tensor("wh0bd", [_E, 512], bf16, kind="ExternalInput")
    wh1_in = nc.dram_tensor("wh1bd", [_E, 512], bf16, kind="ExternalInput")
    b0_in = nc.dram_tensor("bias0", [_E, 4], f32, kind="ExternalInput")
    b1_in = nc.dram_tensor("bias1", [_E, 4], f32, kind="ExternalInput")
    out_d = nc.dram_tensor("outp", [128, 4], f32, kind="ExternalOutput")
    cs_d = nc.dram_tensor("colsum", [1, NGLOB], f32, kind="ExternalOutput")
    if dbg:
        anb_d = nc.dram_tensor("anb", [128, 4 * NLOC], f32,
                               kind="ExternalOutput")

    # DRAM scratch for the AllGather
    ag_in_d = nc.dram_tensor("ag_in", [128, NLOC], bf16)
    ag_out_d = nc.dram_tensor("ag_out", [_NCORES * 128, NLOC], bf16,
                              addr_space="Shared")

    with tile.TileContext(nc) as tc:
        with (
            tc.tile_pool(name="consts", bufs=1) as consts,
            tc.tile_pool(name="bigbuf", bufs=1) as bigbuf,
            tc.tile_pool(name="state", bufs=1) as state,
            tc.tile_pool(name="small", bufs=3) as small,
            tc.tile_pool(name="nrm", bufs=1) as nrm,
            tc.tile_pool(name="sg", bufs=4) as sgp,
        ):
            dma = nc.sync.dma_start

            # ---------- load constants / inputs ----------
            x_sb = bigbuf.tile([_D, TS], bf16, tag="x")
            xr_sb = bigbuf.tile([_D, TS], bf16, tag="xr")
            dma(out=x_sb[:], in_=x_in.ap())
            dma(out=xr_sb[:], in_=xr_in.ap())
            wi0_sb = consts.tile([_D, 512], bf16)
            wi1_sb = consts.tile([_E, 512], bf16)
            wh0_sb = consts.tile([_E, 512], bf16)
            wh1_sb = consts.tile([_E, 512], bf16)
            b0_sb = consts.tile([_E, 4], f32)
            b1_sb = consts.tile([_E, 4], f32)
            for sb, di in ((wi0_sb, wi0_in), (wi1_sb, wi1_in),
                           (wh0_sb, wh0_in), (wh1_sb, wh1_in),
                           (b0_sb, b0_in), (b1_sb, b1_in)):
                dma(out=sb[:], in_=di.ap())
            ident = consts.tile([128, 128], bf16)
            make_identity(nc, ident[:])
            ones_col = consts.tile([128, 1], bf16)
            nc.vector.memset(ones_col[:], 1.0)
            ones_row = consts.tile([1, 128], f32)
            nc.vector.memset(ones_row[:], 1.0)
            ones_rowb = consts.tile([1, 128], bf16)
            nc.vector.memset(ones_rowb[:], 1.0)

            # ---------- window offsets on device (only needs u) ----------
            # o[b,t] = i2[b,t] - t in [-3, 4]; the z2 sampling is then 8
            # one-hot masked shift-accumulates (no gpsimd gather needed)
            ob = consts.tile([1, _BS * T], bf16)
            with tc.tile_pool(name="idxp", bufs=1) as idxp:
                u2 = idxp.tile([1, _BS * T], f32, name="u2t")
                tf = idxp.tile([1, _BS * T], f32, name="tft")
                dma(out=u2[:], in_=u2_in.ap())
                dma(out=tf[:], in_=tw_in.ap())
                ks = idxp.tile([1, _BS * T], f32, name="kst")
                nc.vector.tensor_scalar_mul(ks[:], tf[:], 1.0 / (T - 1))
                # the oracle's (k*s).astype(int32) rounds-to-nearest on
                # neuron, so the carry fires at 0.5 rather than 1.0
                nc.vector.tensor_scalar(ks[:], ks[:], 0.5, None, OP.is_ge)
                nc.vector.tensor_add(ks[:], tf[:], ks[:])      # center
                lo = idxp.tile([1, _BS * T], f32, name="lot")
                nc.vector.tensor_scalar(lo[:], ks[:], -float(_W), 0.0,
                                        OP.add, OP.max)
                hi = idxp.tile([1, _BS * T], f32, name="hit")
                nc.vector.tensor_scalar(hi[:], ks[:], float(_W), float(T),
                                        OP.add, OP.min)
                cnt = idxp.tile([1, _BS * T], f32, name="cntt")
                nc.vector.tensor_sub(cnt[:], hi[:], lo[:])
                nc.vector.tensor_mul(cnt[:], u2[:], cnt[:])    # pr
                fr = idxp.tile([1, _BS * T], f32, name="frt")
                nc.vector.memset(fr[:], 0.0)
                for kth in range(1, 2 * _W + 1):
                    nc.vector.scalar_tensor_tensor(
                        fr[:], cnt[:], float(kth), fr[:], OP.is_ge, OP.add)
                nc.vector.tensor_add(lo[:], lo[:], fr[:])      # i2 (pre-min)
                nc.vector.tensor_scalar_add(hi[:], hi[:], -1.0)
                nc.vector.tensor_tensor(lo[:], lo[:], hi[:], op=OP.min)
                nc.vector.tensor_sub(lo[:], lo[:], tf[:])      # o = i2 - t
                nc.vector.tensor_copy(ob[:], lo[:])
            # broadcast o across partitions (1-contraction matmuls) and
            # build the 8 one-hot masks; overlaps the LSTM epilogue
            ones_rowb0 = consts.tile([1, 128], bf16)
            nc.vector.memset(ones_rowb0[:], 1.0)
            # mask storage is aliased into the XG tiles and H0r, which
            # are dead once the layer-2 recurrence has consumed them

            # ---------- LSTM ----------
            def xg_precompute(ps_big, wiT_sb, K, srcs_f, srcs_b, bias_sb,
                              XGs):
                # XG layout: [128, (t, gate, s)] with gf = GF per step,
                # chunked into NFC tiles so the recurrence can start as
                # soon as chunk 0 is written
                for ch in range(NFC):
                    XG3 = XGs[ch][:].rearrange("p (t gf) -> p t gf", gf=GF)
                    for g in range(4):
                        ps = ps_big.tile([128, FC], f32, tag="xgps")
                        nc.tensor.matmul(
                            out=ps[0:_H, :],
                            lhsT=wiT_sb[0:K, g * 64:(g + 1) * 64],
                            rhs=srcs_f[ch],
                            start=True, stop=True)
                        nc.tensor.matmul(
                            out=ps[_H:128, :],
                            lhsT=wiT_sb[0:K, 256 + g * 64:256 + (g + 1) * 64],
                            rhs=srcs_b[ch],
                            start=True, stop=True)
                        dst = XG3[:, :, g * S:(g + 1) * S]
                        nc.scalar.activation(
                            dst,
                            ps[:].rearrange("p (t s) -> p t s", s=S),
                            AF.Identity, bias=bias_sb[:, g:g + 1])

            def recurrence(ps_gate, wh_sb, XGs, Ht, Hr=None, lyr=0):
                # Cell state via tensor_tensor_scan: we track ct = c/2, so
                #   ct_t = sf * ct_{t-1} + m1',  m1' = (u-0.5)*si
                #        (= [sf*c + si*tanh(g)] / 2 since tanh(g) = 2u-1)
                #   tanh(c) = Tanh(2*ct)  (ACT scale=2)
                # The scan's free layout interleaves a reset slot (d0=0,
                # d1=ct_prev) and an update slot (d0=sf, d1=m1') per stream.
                # sigma writes all gates strided into sgx (odd cols); even
                # cols stay 0 so sgx[8:16] is [0 sf 0 sf ...] = the scan d0.
                # R tiles: scan(t) writes [e0 n0 e1 n1 ...] into R[t%2][0:8]
                # (n_s = new ct at col 2s+1); m1'(t+1) then lands at cols
                # {2,4,6,8} of the same tile, so R[t%2][1:9] is the next d1.
                Hsts = [state.tile([128, S], bf16, tag=f"hst{lyr}{i}",
                                   name=f"hst{lyr}{i}") for i in range(3)]
                Rs = [state.tile([128, 2 * S + 2], f32, tag=f"r{lyr}{i}",
                                 name=f"r{lyr}{i}") for i in range(2)]
                Tc = [state.tile([128, S], bf16, tag=f"tc{lyr}{i}",
                                 name=f"tc{lyr}{i}") for i in range(2)]
                Sgx = [state.tile([128, 2 * GF], f32, tag=f"sg{lyr}{i}",
                                  name=f"sg{lyr}{i}") for i in range(2)]
                for i in range(3):
                    nc.vector.memset(Hsts[i][:], 0.0)
                for i in range(2):
                    nc.vector.memset(Rs[i][:], 0.0)
                    nc.vector.memset(Sgx[i][:], 0.0)

                def sview(sgx, g):
                    # [128, S, 1] view of gate g's columns {2*(g*S+s)+1}
                    v = sgx[:].rearrange("p (c z) -> p c z", z=2)
                    return v[:, g * S:(g + 1) * S, 1:2]

                def emit_copies(t, Hst):
                    rt = T - 1 - t
                    nc.gpsimd.tensor_copy(Ht[0:_H, t * S:(t + 1) * S],
                                          Hst[0:_H, :])
                    nc.gpsimd.tensor_copy(Ht[_H:128, rt * S:(rt + 1) * S],
                                          Hst[_H:128, :])
                    if Hr is not None:
                        nc.gpsimd.tensor_copy(Hr[0:_H, rt * S:(rt + 1) * S],
                                              Hst[0:_H, :])
                        nc.gpsimd.tensor_copy(Hr[_H:128, t * S:(t + 1) * S],
                                              Hst[_H:128, :])

                for t in range(T):
                    Hprev = Hsts[(t + 2) % 3]
                    Hst = Hsts[t % 3]
                    Ra = Rs[t % 2]
                    Rb = Rs[(t + 1) % 2]
                    tc = Tc[t % 2]
                    sgx = Sgx[t % 2]
                    gb = ps_gate.tile([128, GF], f32, tag="gates")
                    xgsl = XGs[t // TCH][:, (t % TCH) * GF:
                                         (t % TCH + 1) * GF]
                    # inject xg via identity matmul (clears has_written)
                    nc.tensor.matmul(out=gb[:], lhsT=ident[:],
                                     rhs=xgsl,
                                     start=True, stop=False,
                                     skip_group_check=True)
                    for g in (1, 2, 3, 0):
                        nc.tensor.matmul(
                            out=gb[:, g * S:(g + 1) * S],
                            lhsT=wh_sb[:, g * 128:(g + 1) * 128],
                            rhs=Hprev[:],
                            start=False, stop=(g == 0),
                            skip_group_check=True)
                    # sigmoid covers all gates (g-gate weights x2 on host,
                    # so its col holds u = sigmoid(2g)); split so the
                    # (i,f,g) part fires before the o-gate matmul lands;
                    # output is strided into odd cols of sgx
                    gb3 = gb[:, S:GF].rearrange("p (c z) -> p c z", z=1)
                    sgv = sgx[:].rearrange("p (c z) -> p c z", z=2)
                    nc.scalar.activation(sgv[:, S:GF, 1:2], gb3, AF.Sigmoid)
                    gb3o = gb[:, 0:S].rearrange("p (c z) -> p c z", z=1)
                    nc.scalar.activation(sgv[:, 0:S, 1:2], gb3o, AF.Sigmoid)
                    # m1' = (u - 0.5) * si into Rb cols {2,4,6,8}
                    m1o = Rb[:, 2:2 * S + 2].rearrange(
                        "p (s z) -> p s z", z=2)[:, :, 0:1]
                    nc.vector.scalar_tensor_tensor(
                        m1o, sview(sgx, 3), 0.5, sview(sgx, 1),
                        OP.subtract, OP.mult)
                    # ct scan: d0 = [0 sf 0 sf ...], d1 = [ct_prev m1' ...]
                    nc.vector.tensor_tensor_scan(
                        Ra[:, 0:2 * S], sgx[:, 4 * S:6 * S],
                        Rb[:, 1:2 * S + 1], 0.0, OP.mult, OP.add)
                    # tanh(c) = Tanh(2*ct) from odd cols of Ra
                    tanh_in = Ra[:, 1:2 * S + 1].rearrange(
                        "p (s z) -> p s z", z=2)[:, :, 0:1]
                    tc3 = tc[:].rearrange("p (s z) -> p s z", z=1)
                    nc.scalar.activation(tc3, tanh_in, AF.Tanh, scale=2.0)
                    hst3 = Hst[:].rearrange("p (s z) -> p s z", z=1)
                    nc.vector.tensor_tensor(hst3, sview(sgx, 0), tc3,
                                            op=OP.mult)
                    emit_copies(t, Hst)

            H0t = bigbuf.tile([128, TS], bf16, tag="h0t")
            H1t = bigbuf.tile([128, TS], bf16, tag="h1t")
            with (
                tc.tile_pool(name="ps_big", bufs=2, space="PSUM") as ps_big,
                tc.tile_pool(name="ps_gate", bufs=3, space="PSUM") as ps_gate,
            ):
                XGs = [bigbuf.tile([128, TCH * GF], bf16, tag=f"xg{ch}",
                                   name=f"xg{ch}")
                       for ch in range(NFC)]
                H0r = bigbuf.tile([128, TS], bf16, tag="h0r")
                srcs_f = [x_sb[:, ch * FC:(ch + 1) * FC] for ch in range(NFC)]
                srcs_b = [xr_sb[:, ch * FC:(ch + 1) * FC] for ch in range(NFC)]
                xg_precompute(ps_big, wi0_sb, _D, srcs_f, srcs_b, b0_sb, XGs)
                recurrence(ps_gate, wh0_sb, XGs, H0t, H0r, lyr=0)

                # layer 2: fwd reads H0t, bwd reads the reversed copy H0r
                # (negative-stride APs are rejected by the BIR verifier)
                srcs_f = [H0t[:, ch * FC:(ch + 1) * FC] for ch in range(NFC)]
                srcs_b = [H0r[:, ch * FC:(ch + 1) * FC] for ch in range(NFC)]
                xg_precompute(ps_big, wi1_sb, _E, srcs_f, srcs_b, b1_sb, XGs)
                recurrence(ps_gate, wh1_sb, XGs, H1t, lyr=1)

            # ---------- normalize + pack An (z1) / Z2 ----------
            An = bigbuf.tile([128, NLOC], bf16, tag="an")
            Z2f = bigbuf.tile([128, NLOC + 8], bf16, tag="z2")
            nc.vector.memset(Z2f[:, 0:4], 0.0)
            nc.vector.memset(Z2f[:, NLOC + 4:NLOC + 8], 0.0)
            Bn = bigbuf.tile([128, NLOC], bf16, tag="bn")
            masks = [XGs[dd // 2][:, (dd % 2) * NLOC:(dd % 2 + 1) * NLOC]
                     for dd in range(2 * _W + 2)]
            osb = H0r[:, 0:NLOC]
            tmpb = H0r[:, NLOC:2 * NLOC]
            with tc.tile_pool(name="ps_ob", bufs=2, space="PSUM") as ps_ob:
                for b in range(_BS):
                    pso = ps_ob.tile([128, T], f32, tag="pso")
                    nc.tensor.matmul(out=pso[:], lhsT=ones_rowb0[:],
                                     rhs=ob[:, b * T:(b + 1) * T],
                                     start=True, stop=True)
                    nc.vector.tensor_copy(osb[:, b * T:(b + 1) * T], pso[:])
            for dd in range(2 * _W + 2):
                nc.vector.tensor_scalar(masks[dd], osb[:],
                                        float(dd - _W), None, OP.is_equal)
            H13 = H1t[:].rearrange("p (t s) -> p t s", s=S)
            n2 = nrm.tile([1, TS], f32, tag="n2")
            sq = bigbuf.tile([128, FC], bf16, tag="sq")
            with (
                tc.tile_pool(name="ps_nrm", bufs=4, space="PSUM") as ps_nrm,
                tc.tile_pool(name="ps_row", bufs=2, space="PSUM") as ps_row,
            ):
                for ch in range(NFC):
                    nc.vector.tensor_mul(sq[:],
                                         H1t[:, ch * FC:(ch + 1) * FC],
                                         H1t[:, ch * FC:(ch + 1) * FC])
                    psn = ps_row.tile([1, FC], f32, tag="psn")
                    nc.tensor.matmul(out=psn[:], lhsT=ones_col[:],
                                     rhs=sq[:], start=True, stop=True)
                    nc.vector.tensor_copy(n2[:, ch * FC:(ch + 1) * FC],
                                          psn[:])
                nc.vector.tensor_scalar_max(n2[:], n2[:], 1e-24)
                lnb = nrm.tile([1, TS], f32, tag="lnb")
                nc.scalar.activation(lnb[:], n2[:], AF.Ln)
                rin = nrm.tile([1, TS], bf16, tag="rin")
                nc.scalar.activation(rin[:], lnb[:], AF.Exp, scale=-0.5)
                # pack Z2 first: the gather + chunked AllGather start as
                # early as possible; An is packed during the collectives
                def pack(streams, ch, psb):
                    ps3 = psb[:].rearrange("p (t s) -> p t s", s=S)
                    h3 = H13[:, ch * TCH:(ch + 1) * TCH, :]
                    for s in streams:
                        if s < _BS:
                            dst = An[:, s * T + ch * TCH:
                                     s * T + (ch + 1) * TCH]
                        else:
                            b = s - _BS
                            dst = Z2f[:, 4 + b * T + ch * TCH:
                                      4 + b * T + (ch + 1) * TCH]
                        nc.vector.tensor_tensor(
                            dst,
                            h3[:, :, s:s + 1].rearrange("p t o -> p (t o)"),
                            ps3[:, :, s:s + 1].rearrange("p t o -> p (t o)"),
                            op=OP.mult)

                def mkpsb(ch):
                    psb = ps_nrm.tile([128, FC], f32, tag="nps")
                    nc.tensor.matmul(out=psb[:], lhsT=ones_rowb[:],
                                     rhs=rin[:, ch * FC:(ch + 1) * FC],
                                     start=True, stop=True)
                    return psb
                psbs = []
                for ch in range(NFC):
                    psb = mkpsb(ch)
                    psbs.append(psb)
                    pack(range(_BS, S), ch, psb)

                # ---------- gather z2 -> Bn: one-hot shift-accumulate ----
                # Bn[:,t] = sum_d mask_d[t] * Z2f[:, 4+t+d]; the clamped
                # index computation guarantees masked-out terms never pick
                # the padding or a neighboring row's data
                nc.vector.tensor_tensor(Bn[:], Z2f[:, 1:NLOC + 1],
                                        masks[0], op=OP.mult)
                for dd in range(1, 2 * _W + 2):
                    nc.vector.tensor_tensor(
                        tmpb, Z2f[:, 1 + dd:NLOC + 1 + dd],
                        masks[dd], op=OP.mult)
                    nc.vector.tensor_add(Bn[:], Bn[:], tmpb)
                dma(out=ag_in_d.ap(), in_=Bn[:])
                nc.gpsimd.collective_compute(
                    "AllGather", OP.bypass,
                    replica_groups=[list(range(_NCORES))],
                    ins=[ag_in_d.ap().opt()],
                    outs=[ag_out_d.ap().opt()])

                # pack An while the collectives run
                for ch in range(NFC):
                    pack(range(_BS), ch, psbs[ch])

            Ball = bigbuf.tile([128, NGLOB], bf16, tag="ball")
            for jj in range(_NCORES):
                dma(out=Ball[:, jj * NLOC:(jj + 1) * NLOC],
                    in_=ag_out_d.ap()[jj * 128:(jj + 1) * 128, :])

            # ---------- output partials ----------
            outp = consts.tile([128, 4], f32)
            nc.vector.memset(outp[:], 0.0)

            with (
                tc.tile_pool(name="ps_d", bufs=1, space="PSUM") as ps_d,
                tc.tile_pool(name="ps_s", bufs=2, space="PSUM") as ps_s,
                tc.tile_pool(name="ps_c", bufs=1, space="PSUM") as ps_c,
            ):
                # diag: sum_i <An_i, Bn_i>
                dg = bigbuf.tile([128, NLOC], bf16, tag="dg")
                nc.vector.tensor_mul(dg[:], An[:], Bn[:])
                ndc = (NLOC + 511) // 512
                psd = ps_d.tile([1, 512], f32, tag="psd")
                for ch in range(ndc):
                    nc.tensor.matmul(out=psd[:], lhsT=ones_col[:],
                                     rhs=dg[:, ch * 512:(ch + 1) * 512],
                                     start=(ch == 0), stop=(ch == ndc - 1))
                nc.vector.reduce_sum(outp[0:1, 2:3], psd[:],
                                     axis=mybir.AxisListType.X)

                # one-pass row+col logsumexp sweep ([128,1024] exp
                # blocks; row sums on DVE, col sums via ones-matmuls)
                CP = 2 * CC
                NCP = NGLOB // CP
                rows = bigbuf.tile([128, NRC * NCP], f32, tag="rows")
                csum = nrm.tile([1, NGLOB], f32, tag="csum")
                for cb in range(NCP):
                    csp0 = ps_c.tile([1, CC], f32, tag="csp0")
                    csp1 = ps_c.tile([1, CC], f32, tag="csp1")
                    for rc in range(NRC):
                        ps = ps_s.tile([128, CP], f32, tag="sps")
                        nc.tensor.matmul(
                            out=ps[:, 0:CC],
                            lhsT=An[:, rc * 128:(rc + 1) * 128],
                            rhs=Ball[:, cb * CP:cb * CP + CC],
                            start=True, stop=True)
                        nc.tensor.matmul(
                            out=ps[:, CC:CP],
                            lhsT=An[:, rc * 128:(rc + 1) * 128],
                            rhs=Ball[:, cb * CP + CC:(cb + 1) * CP],
                            start=True, stop=True)
                        eb = sgp.tile([128, CP], bf16, tag="eb")
                        nc.scalar.activation(
                            eb[:], ps[:], AF.Exp, scale=1.0 / _TEMP)
                        nc.vector.reduce_sum(
                            rows[:, rc * NCP + cb:rc * NCP + cb + 1],
                            eb[:], axis=mybir.AxisListType.X)
                        nc.tensor.matmul(
                            out=csp0[:], lhsT=ones_col[:], rhs=eb[:, 0:CC],
                            start=(rc == 0), stop=(rc == NRC - 1))
                        nc.tensor.matmul(
                            out=csp1[:], lhsT=ones_col[:], rhs=eb[:, CC:CP],
                            start=(rc == 0), stop=(rc == NRC - 1))
                    nc.vector.tensor_copy(csum[:, cb * CP:cb * CP + CC],
                                          csp0[:])
                    nc.vector.tensor_copy(
                        csum[:, cb * CP + CC:(cb + 1) * CP], csp1[:])

                tot = small.tile([128, NRC], f32, tag="tot")
                for rc in range(NRC):
                    nc.vector.reduce_sum(tot[:, rc:rc + 1],
                                         rows[:, rc * NCP:(rc + 1) * NCP],
                                         axis=mybir.AxisListType.X)
                lse = small.tile([128, NRC], f32, tag="lse")
                nc.scalar.activation(lse[:], tot[:], AF.Ln)
                nc.vector.reduce_sum(outp[:, 0:1], lse[:],
                                     axis=mybir.AxisListType.X)

            dma(out=out_d.ap(), in_=outp[:])
            dma(out=cs_d.ap(), in_=csum[:])
            if dbg:
                anb32 = bigbuf.tile([128, NLOC], f32, tag="anb32")
                for i, src in enumerate((An, Bn)):
                    nc.vector.tensor_copy(anb32[:], src[:])
                    dma(out=anb_d.ap()[:, i * NLOC:(i + 1) * NLOC],
                        in_=anb32[:])
                nc.vector.tensor_copy(anb32[:], Z2f[:, 4:NLOC + 4])
                dma(out=anb_d.ap()[:, 2 * NLOC:3 * NLOC], in_=anb32[:])
                nc.vector.tensor_copy(anb32[:], H1t[:, 0:NLOC])
                dma(out=anb_d.ap()[:, 3 * NLOC:4 * NLOC], in_=anb32[:])

    nc.compile()
    return nc


def _host_prep(x1, x2, u, wih0, whh0, bih0, bhh0, wih1, whh1, bih1, bhh1, T):
    """Build per-core input maps (all host work is pure data layout)."""
    import ml_dtypes
    bf16 = ml_dtypes.bfloat16

    # g-gate (kernel position _GTANH) weights/bias are pre-scaled by 2 so
    # a single sigmoid gives u = sigmoid(2g), tanh(g) = 2u - 1
    def gate_stack_T(w):
        # w: [2, 256, K] -> [K, 512]; cols = dir*256 + gperm_gate*64 + j
        K = w.shape[2]
        out = np.empty((K, 512), np.float32)
        for d in range(2):
            for gi, g in enumerate(_GPERM):
                sc = 2.0 if gi == _GTANH else 1.0
                out[:, d * 256 + gi * 64:d * 256 + (gi + 1) * 64] = \
                    sc * w[d, g * 64:(g + 1) * 64, :].T
        return out.astype(bf16)

    def blockdiag(w):
        # w: [2, 256, H] -> [128, 512]; per new-gate [128,128] block-diag
        out = np.zeros((128, 512), np.float32)
        for gi, g in enumerate(_GPERM):
            sc = 2.0 if gi == _GTANH else 1.0
            out[0:_H, gi * 128:gi * 128 + 64] = \
                sc * w[0, g * 64:(g + 1) * 64, :].T
            out[_H:128, gi * 128 + 64:(gi + 1) * 128] = \
                sc * w[1, g * 64:(g + 1) * 64, :].T
        return out.astype(bf16)

    def biases(bi, bh):
        b = bi + bh  # [2, 256]
        out = np.empty((128, 4), np.float32)
        for gi, g in enumerate(_GPERM):
            sc = 2.0 if gi == _GTANH else 1.0
            out[0:_H, gi] = sc * b[0, g * 64:(g + 1) * 64]
            out[_H:128, gi] = sc * b[1, g * 64:(g + 1) * 64]
        return out

    twt = np.tile(np.arange(T, dtype=np.float32), _BS)[None, :]

    shared = {
        "wi0T": np.ascontiguousarray(gate_stack_T(wih0)),
        "wi1T": np.ascontiguousarray(gate_stack_T(wih1)),
        "wh0bd": blockdiag(whh0),
        "wh1bd": blockdiag(whh1),
        "bias0": biases(bih0, bhh0),
        "bias1": biases(bih1, bhh1),
        "tw2": np.ascontiguousarray(twt),
    }
    in_maps = []
    for k in range(_NCORES):
        rows = [x1[2 * k, :T], x1[2 * k + 1, :T], x2[2 * k, :T],
                x2[2 * k + 1, :T]]
        arr = np.stack(rows, axis=2)            # [T, D, S]
        xc = np.ascontiguousarray(
            arr.transpose(1, 0, 2).reshape(_D, T * _S)).astype(bf16)
        xr = np.ascontiguousarray(
            arr[::-1].transpose(1, 0, 2).reshape(_D, T * _S)).astype(bf16)
        m = dict(shared)
        m["x_cat"] = xc
        m["x_rev"] = xr
        m["u2"] = np.ascontiguousarray(
            u[2 * k:2 * k + 2, :T].reshape(1, -1))
        in_maps.append(m)
    return in_maps


def _run(inputs, T=_T, trace=False, dbg=False):
    from concourse import bass_utils
    key = (T, dbg)
    if key not in _cache:
        _cache[key] = _build(T, dbg)
    nc = _cache[key]
    in_maps = _host_prep(T=T, **inputs)
    res = bass_utils.run_bass_kernel_spmd(
        nc, in_maps, core_ids=list(range(_NCORES)), trace=trace)
    N = _NCORES * _BS * T
    R = sum(float(r["outp"][:, 0].sum()) for r in res.results)
    Draw = sum(float(r["outp"][0, 2]) for r in res.results)
    colsum = np.zeros(N, np.float64)
    for r in res.results:
        colsum += np.asarray(r["colsum"][0], np.float64)
    C = float(np.log(colsum).sum())
    Dg = Draw / _TEMP
    loss = -((Dg - R) / N + (Dg - C) / N)
    return np.float32(loss), res


def kernel(**inputs):
    loss, _ = _run(inputs)
    return np.asarray(loss, dtype=np.float32)



# revision 6
# speedup vs baseline: 1.1091x; 1.0027x over previous
"""Trainium2 Bass kernel for nn_MIPS_74904229642848 (v3).

Pipeline (8 NeuronCores, SPMD, batch-sharded 2 rows/core, S=4 streams/core):
  1. 2-layer bidirectional LSTM, all-bf16 matmuls. Per step: one identity
     matmul injects the precomputed input-gate terms into PSUM (chunked so
     each recurrence starts after its first xg chunk), four bf16 block-diag
     recurrence matmuls accumulate on top. The g-gate weights/bias are
     pre-scaled by 2 on the host so ONE sigmoid over all four gates covers
     the tanh as well (tanh(g) = 2*sigmoid(2g)-1); the cell update is two
     fused scalar_tensor_tensor ops on DVE (m1' = (u-.5)*si;
     c = 2*m1' + m2) with m2 = sf*c on GpSimd, c in SBUF, tanh(c) on ACT,
     h = so*tanh(c) on DVE into a triple-buffered Hst.
  2. L2 normalization via ln/exp rsqrt (no Newton), fused scale+pack.
  3. Windowed index sampling of z2 via indirect DMA (bf16).
  4. AllGather of the B-side embeddings only (bf16).
  5. One-pass logits sweep: bf16 sim matmul blocks, exp (ACT, accum_out
     gives row sums), ones-matmul accumulates column sums in PSUM across
     row blocks. Row-lse finished on device; per-core column-sum partials
     shipped to the host, which does the final ln+sum combine.
"""

import numpy as np

_D, _E, _H, _B, _W = 64, 128, 64, 16, 3
_T = 512
_TEMP = 0.05
_NCORES = 8
_BS = _B // _NCORES          # batch rows per core
_S = 2 * _BS                 # streams per core: (x1,b0),(x1,b1),(x2,b0),(x2,b1)
_GF = 4 * _S                 # gate-block width per step

# torch gate order i,f,g,o -> kernel order o,i,f,g (tanh block last)
_GPERM = [3, 0, 1, 2]
_GTANH = 3                   # index of the g gate in kernel order

_cache = {}


def _build(T, dbg=False):
    import concourse.bass as bass
    import concourse.mybir as mybir
    import concourse.tile as tile
    from concourse import bacc, library_config
    from concourse.masks import make_identity

    f32 = mybir.dt.float32
    bf16 = mybir.dt.bfloat16
    i32 = mybir.dt.int32
    AF = mybir.ActivationFunctionType
    OP = mybir.AluOpType

    S = _S
    GF = _GF
    TS = T * S
    NLOC = _BS * T
    NGLOB = _NCORES * NLOC
    FC = min(512, TS)            # xg free chunk
    NFC = TS // FC
    TCH = FC // S                # timesteps per xg chunk
    CC = min(512, NGLOB)         # logits col chunk
    NCC = NGLOB // CC
    NRC = (NLOC + 127) // 128    # logits row chunks (M=128)
    NTC = T // 128               # transpose chunks per stream
    NUC = T // 128

    nc = bacc.Bacc("TRN2", target_bir_lowering=False, debug=False,
                   num_devices=_NCORES)

    # ---- I/O ----
    x_in = nc.dram_tensor("x_cat", [_D, TS], bf16, kind="ExternalInput")
    xr_in = nc.dram_tensor("x_rev", [_D, TS], bf16, kind="ExternalInput")
    u2_in = nc.dram_tensor("u2", [1, _BS * T], f32, kind="ExternalInput")
    tw_in = nc.dram_tensor("tw2", [1, _BS * T], f32, kind="ExternalInput")
    wi0_in = nc.dram_tensor("wi0T", [_D, 512], bf16, kind="ExternalInput")
    wi1_in = nc.dram_tensor("wi1T", [_E, 512], bf16, kind="ExternalInput")
    wh0_in = nc.dram_tensor("wh0bd", [_E, 512], bf16, kind="ExternalInput")
    wh1_in = nc.dram_tensor("wh1bd", [_E, 512], bf16, kind="ExternalInput")
    b0_in = nc.dram_tensor("bias0", [_E, 4], f32, kind="ExternalInput")
    b1_in = nc.dram_tensor("bias1", [_E, 4], f32, kind="ExternalInput")
    out_d = nc.dram_tensor("outp", [128, 4], f32, kind="ExternalOutput")
    cs_d = nc.dram_tensor("colsum", [1, NGLOB], f32, kind="ExternalOutput")
    if dbg:
        anb_d = nc.dram_tensor("anb", [128, 4 * NLOC], f32,
                               kind="ExternalOutput")

    # DRAM scratch for the AllGather
    ag_in_d = nc.dram_tensor("ag_in", [128, NLOC], bf16)
    ag_out_d = nc.dram_tensor("ag_out", [_NCORES * 128, NLOC], bf16,
                              addr_space="Shared")

    with tile.TileContext(nc) as tc:
        with (
            tc.tile_pool(name="consts", bufs=1) as consts,
            tc.tile_pool(name="bigbuf", bufs=1) as bigbuf,
            tc.tile_pool(name="state", bufs=1) as state,
            tc.tile_pool(name="small", bufs=3) as small,
            tc.tile_pool(name="nrm", bufs=1) as nrm,
            tc.tile_pool(name="sg", bufs=4) as sgp,
        ):
            dma = nc.sync.dma_start

            # ---------- load constants / inputs ----------
            x_sb = bigbuf.tile([_D, TS], bf16, tag="x")
            xr_sb = bigbuf.tile([_D, TS], bf16, tag="xr")
            dma(out=x_sb[:], in_=x_in.ap())
            dma(out=xr_sb[:], in_=xr_in.ap())
            wi0_sb = consts.tile([_D, 512], bf16)
            wi1_sb = consts.tile([_E, 512], bf16)
            wh0_sb = consts.tile([_E, 512], bf16)
            wh1_sb = consts.tile([_E, 512], bf16)
            b0_sb = consts.tile([_E, 4], f32)
            b1_sb = consts.tile([_E, 4], f32)
            for sb, di in ((wi0_sb, wi0_in), (wi1_sb, wi1_in),
                           (wh0_sb, wh0_in), (wh1_sb, wh1_in),
                           (b0_sb, b0_in), (b1_sb, b1_in)):
                dma(out=sb[:], in_=di.ap())
            ident = consts.tile([128, 128], bf16)
            make_identity(nc, ident[:])
            ones_col = consts.tile([128, 1], bf16)
            nc.vector.memset(ones_col[:], 1.0)
            ones_row = consts.tile([1, 128], f32)
            nc.vector.memset(ones_row[:], 1.0)
            ones_rowb = consts.tile([1, 128], bf16)
            nc.vector.memset(ones_rowb[:], 1.0)

            # ---------- window offsets on device (only needs u) ----------
            # o[b,t] = i2[b,t] - t in [-3, 4]; the z2 sampling is then 8
            # one-hot masked shift-accumulates (no gpsimd gather needed)
            # rows b live at partitions 0 and 32 (legal matmul rhs bases);
            # the ops run on the full [64, T] view so they stream 512-wide
            ob = consts.tile([64, T], bf16)
            with tc.tile_pool(name="idxp", bufs=1) as idxp:
                u2 = idxp.tile([64, T], f32, name="u2t")
                tf = idxp.tile([64, T], f32, name="tft")
                for b in range(_BS):
                    dma(out=u2[32 * b:32 * b + 1, :],
                        in_=u2_in.ap()[0:1, b * T:(b + 1) * T])
                    dma(out=tf[32 * b:32 * b + 1, :],
                        in_=tw_in.ap()[0:1, b * T:(b + 1) * T])
                ks = idxp.tile([64, T], f32, name="kst")
                nc.vector.tensor_scalar_mul(ks[:], tf[:], 1.0 / (T - 1))
                # the oracle's (k*s).astype(int32) rounds-to-nearest on
                # neuron, so the carry fires at 0.5 rather than 1.0
                nc.vector.tensor_scalar(ks[:], ks[:], 0.5, None, OP.is_ge)
                nc.vector.tensor_add(ks[:], tf[:], ks[:])      # center
                lo = idxp.tile([64, T], f32, name="lot")
                nc.vector.tensor_scalar(lo[:], ks[:], -float(_W), 0.0,
                                        OP.add, OP.max)
                hi = idxp.tile([64, T], f32, name="hit")
                nc.vector.tensor_scalar(hi[:], ks[:], float(_W), float(T),
                                        OP.add, OP.min)
                cnt = idxp.tile([64, T], f32, name="cntt")
                nc.vector.tensor_sub(cnt[:], hi[:], lo[:])
                nc.vector.tensor_mul(cnt[:], u2[:], cnt[:])    # pr
                fr = idxp.tile([64, T], f32, name="frt")
                nc.vector.memset(fr[:], 0.0)
                for kth in range(1, 2 * _W + 1):
                    nc.vector.scalar_tensor_tensor(
                        fr[:], cnt[:], float(kth), fr[:], OP.is_ge, OP.add)
                nc.vector.tensor_add(lo[:], lo[:], fr[:])      # i2 (pre-min)
                nc.vector.tensor_scalar_add(hi[:], hi[:], -1.0)
                nc.vector.tensor_tensor(lo[:], lo[:], hi[:], op=OP.min)
                nc.vector.tensor_sub(lo[:], lo[:], tf[:])      # o = i2 - t
                nc.vector.tensor_copy(ob[:], lo[:])
            # broadcast o across partitions (1-contraction matmuls) and
            # build the 8 one-hot masks; overlaps the LSTM epilogue
            ones_rowb0 = consts.tile([64, 128], bf16)
            nc.vector.memset(ones_rowb0[:], 1.0)
            # mask storage is aliased into the XG tiles and H0r, which
            # are dead once the layer-2 recurrence has consumed them

            # ---------- LSTM ----------
            def xg_precompute(ps_big, wiT_sb, K, srcs_f, srcs_b, bias_sb,
                              XGs):
                # XG layout: [128, (t, gate, s)] with gf = GF per step,
                # chunked into NFC tiles so the recurrence can start as
                # soon as chunk 0 is written
                for ch in range(NFC):
                    XG3 = XGs[ch][:].rearrange("p (t gf) -> p t gf", gf=GF)
                    for g in range(4):
                        ps = ps_big.tile([128, FC], f32, tag="xgps")
                        nc.tensor.matmul(
                            out=ps[0:_H, :],
                            lhsT=wiT_sb[0:K, g * 64:(g + 1) * 64],
                            rhs=srcs_f[ch],
                            start=True, stop=True)
                        nc.tensor.matmul(
                            out=ps[_H:128, :],
                            lhsT=wiT_sb[0:K, 256 + g * 64:256 + (g + 1) * 64],
                            rhs=srcs_b[ch],
                            start=True, stop=True)
                        dst = XG3[:, :, g * S:(g + 1) * S]
                        nc.scalar.activation(
                            dst,
                            ps[:].rearrange("p (t s) -> p t s", s=S),
                            AF.Identity, bias=bias_sb[:, g:g + 1])

            def recurrence(ps_gate, wh_sb, XGs, Ht, Hr=None, lyr=0):
                # Cell state via tensor_tensor_scan: we track ct = c/2, so
                #   ct_t = sf * ct_{t-1} + m1',  m1' = (u-0.5)*si
                #        (= [sf*c + si*tanh(g)] / 2 since tanh(g) = 2u-1)
                #   tanh(c) = Tanh(2*ct)  (ACT scale=2)
                # The scan's free layout interleaves a reset slot (d0=0,
                # d1=ct_prev) and an update slot (d0=sf, d1=m1') per stream.
                # sigma writes all gates strided into sgx (odd cols); even
                # cols stay 0 so sgx[8:16] is [0 sf 0 sf ...] = the scan d0.
                # R tiles: scan(t) writes [e0 n0 e1 n1 ...] into R[t%2][0:8]
                # (n_s = new ct at col 2s+1); m1'(t+1) then lands at cols
                # {2,4,6,8} of the same tile, so R[t%2][1:9] is the next d1.
                Hsts = [state.tile([128, S], bf16, tag=f"hst{lyr}{i}",
                                   name=f"hst{lyr}{i}") for i in range(3)]
                Rs = [state.tile([128, 2 * S + 2], f32, tag=f"r{lyr}{i}",
                                 name=f"r{lyr}{i}") for i in range(2)]
                Tc = [state.tile([128, S], bf16, tag=f"tc{lyr}{i}",
                                 name=f"tc{lyr}{i}") for i in range(2)]
                Sgx = [state.tile([128, 2 * GF], f32, tag=f"sg{lyr}{i}",
                                  name=f"sg{lyr}{i}") for i in range(2)]
                for i in range(3):
                    nc.vector.memset(Hsts[i][:], 0.0)
                for i in range(2):
                    nc.vector.memset(Rs[i][:], 0.0)
                    nc.vector.memset(Sgx[i][:], 0.0)

                def sview(sgx, g):
                    # [128, S, 1] view of gate g's columns {2*(g*S+s)+1}
                    v = sgx[:].rearrange("p (c z) -> p c z", z=2)
                    return v[:, g * S:(g + 1) * S, 1:2]

                def emit_copies(t, Hst):
                    # Ht copies on DVE right after h (in-order, no sem);
                    # Hr copies on Pool read Ht, so the h-write never
                    # carries a Pool anti-dependency wait
                    rt = T - 1 - t
                    nc.vector.tensor_copy(Ht[0:_H, t * S:(t + 1) * S],
                                          Hst[0:_H, :])
                    nc.vector.tensor_copy(Ht[_H:128, rt * S:(rt + 1) * S],
                                          Hst[_H:128, :])
                    if Hr is not None:
                        nc.gpsimd.tensor_copy(
                            Hr[0:_H, rt * S:(rt + 1) * S],
                            Ht[0:_H, t * S:(t + 1) * S])
                        nc.gpsimd.tensor_copy(
                            Hr[_H:128, t * S:(t + 1) * S],
                            Ht[_H:128, rt * S:(rt + 1) * S])

                for t in range(T):
                    Hprev = Hsts[(t + 2) % 3]
                    Hst = Hsts[t % 3]
                    Ra = Rs[t % 2]
                    Rb = Rs[(t + 1) % 2]
                    tc = Tc[t % 2]
                    sgx = Sgx[t % 2]
                    gb = ps_gate.tile([128, GF], f32, tag="gates")
                    xgsl = XGs[t // TCH][:, (t % TCH) * GF:
                                         (t % TCH + 1) * GF]
                    # inject xg via identity matmul (clears has_written)
                    nc.tensor.matmul(out=gb[:], lhsT=ident[:],
                                     rhs=xgsl,
                                     start=True, stop=False,
                                     skip_group_check=True)
                    for g in (1, 2, 3, 0):
                        nc.tensor.matmul(
                            out=gb[:, g * S:(g + 1) * S],
                            lhsT=wh_sb[:, g * 128:(g + 1) * 128],
                            rhs=Hprev[:],
                            start=False, stop=(g == 0),
                            skip_group_check=True)
                    # sigmoid covers all gates (g-gate weights x2 on host,
                    # so its col holds u = sigmoid(2g)); split so the
                    # (i,f,g) part fires before the o-gate matmul lands;
                    # output is strided into odd cols of sgx
                    gb3 = gb[:, S:GF].rearrange("p (c z) -> p c z", z=1)
                    sgv = sgx[:].rearrange("p (c z) -> p c z", z=2)
                    nc.scalar.activation(sgv[:, S:GF, 1:2], gb3, AF.Sigmoid)
                    gb3o = gb[:, 0:S].rearrange("p (c z) -> p c z", z=1)
                    nc.scalar.activation(sgv[:, 0:S, 1:2], gb3o, AF.Sigmoid)
                    # m1' = (u - 0.5) * si into Rb cols {2,4,6,8}
                    m1o = Rb[:, 2:2 * S + 2].rearrange(
                        "p (s z) -> p s z", z=2)[:, :, 0:1]
                    nc.vector.scalar_tensor_tensor(
                        m1o, sview(sgx, 3), 0.5, sview(sgx, 1),
                        OP.subtract, OP.mult)
                    # ct scan: d0 = [0 sf 0 sf ...], d1 = [ct_prev m1' ...]
                    nc.vector.tensor_tensor_scan(
                        Ra[:, 0:2 * S], sgx[:, 4 * S:6 * S],
                        Rb[:, 1:2 * S + 1], 0.0, OP.mult, OP.add)
                    # tanh(c) = Tanh(2*ct) from odd cols of Ra
                    tanh_in = Ra[:, 1:2 * S + 1].rearrange(
                        "p (s z) -> p s z", z=2)[:, :, 0:1]
                    tc3 = tc[:].rearrange("p (s z) -> p s z", z=1)
                    nc.scalar.activation(tc3, tanh_in, AF.Tanh, scale=2.0)
                    hst3 = Hst[:].rearrange("p (s z) -> p s z", z=1)
                    nc.vector.tensor_tensor(hst3, sview(sgx, 0), tc3,
                                            op=OP.mult)
                    emit_copies(t, Hst)

            H0t = bigbuf.tile([128, TS], bf16, tag="h0t")
            H1t = bigbuf.tile([128, TS], bf16, tag="h1t")
            with (
                tc.tile_pool(name="ps_big", bufs=2, space="PSUM") as ps_big,
                tc.tile_pool(name="ps_gate", bufs=3, space="PSUM") as ps_gate,
            ):
                XGs = [bigbuf.tile([128, TCH * GF], bf16, tag=f"xg{ch}",
                                   name=f"xg{ch}")
                       for ch in range(NFC)]
                H0r = bigbuf.tile([128, TS], bf16, tag="h0r")
                srcs_f = [x_sb[:, ch * FC:(ch + 1) * FC] for ch in range(NFC)]
                srcs_b = [xr_sb[:, ch * FC:(ch + 1) * FC] for ch in range(NFC)]
                xg_precompute(ps_big, wi0_sb, _D, srcs_f, srcs_b, b0_sb, XGs)
                recurrence(ps_gate, wh0_sb, XGs, H0t, H0r, lyr=0)

                # layer 2: fwd reads H0t, bwd reads the reversed copy H0r
                # (negative-stride APs are rejected by the BIR verifier)
                srcs_f = [H0t[:, ch * FC:(ch + 1) * FC] for ch in range(NFC)]
                srcs_b = [H0r[:, ch * FC:(ch + 1) * FC] for ch in range(NFC)]
                xg_precompute(ps_big, wi1_sb, _E, srcs_f, srcs_b, b1_sb, XGs)
                recurrence(ps_gate, wh1_sb, XGs, H1t, lyr=1)

            # ---------- normalize + pack An (z1) / Z2 ----------
            An = bigbuf.tile([128, NLOC], bf16, tag="an")
            Z2f = bigbuf.tile([128, NLOC + 8], bf16, tag="z2")
            nc.vector.memset(Z2f[:, 0:4], 0.0)
            nc.vector.memset(Z2f[:, NLOC + 4:NLOC + 8], 0.0)
            Bn = bigbuf.tile([128, NLOC], bf16, tag="bn")
            masks = [XGs[dd // 2][:, (dd % 2) * NLOC:(dd % 2 + 1) * NLOC]
                     for dd in range(2 * _W + 2)]
            osb = H0r[:, 0:NLOC]
            tmpb = H0r[:, NLOC:2 * NLOC]
            with tc.tile_pool(name="ps_ob", bufs=2, space="PSUM") as ps_ob:
                for b in range(_BS):
                    pso = ps_ob.tile([128, T], f32, tag="pso")
                    nc.tensor.matmul(out=pso[:],
                                     lhsT=ones_rowb0[32 * b:32 * b + 1, :],
                                     rhs=ob[32 * b:32 * b + 1, :],
                                     start=True, stop=True)
                    nc.vector.tensor_copy(osb[:, b * T:(b + 1) * T], pso[:])
            for dd in range(2 * _W + 2):
                nc.vector.tensor_scalar(masks[dd], osb[:],
                                        float(dd - _W), None, OP.is_equal)
            H13 = H1t[:].rearrange("p (t s) -> p t s", s=S)
            n2 = nrm.tile([1, TS], f32, tag="n2")
            sq = bigbuf.tile([128, FC], bf16, tag="sq")
            with (
                tc.tile_pool(name="ps_nrm", bufs=4, space="PSUM") as ps_nrm,
                tc.tile_pool(name="ps_row", bufs=2, space="PSUM") as ps_row,
            ):
                for ch in range(NFC):
                    nc.vector.tensor_mul(sq[:],
                                         H1t[:, ch * FC:(ch + 1) * FC],
                                         H1t[:, ch * FC:(ch + 1) * FC])
                    psn = ps_row.tile([1, FC], f32, tag="psn")
                    nc.tensor.matmul(out=psn[:], lhsT=ones_col[:],
                                     rhs=sq[:], start=True, stop=True)
                    nc.vector.tensor_copy(n2[:, ch * FC:(ch + 1) * FC],
                                          psn[:])
                nc.vector.tensor_scalar_max(n2[:], n2[:], 1e-24)
                lnb = nrm.tile([1, TS], f32, tag="lnb")
                nc.scalar.activation(lnb[:], n2[:], AF.Ln)
                rin = nrm.tile([1, TS], bf16, tag="rin")
                nc.scalar.activation(rin[:], lnb[:], AF.Exp, scale=-0.5)
                # pack Z2 first: the gather + chunked AllGather start as
                # early as possible; An is packed during the collectives
                def pack(streams, ch, psb):
                    ps3 = psb[:].rearrange("p (t s) -> p t s", s=S)
                    h3 = H13[:, ch * TCH:(ch + 1) * TCH, :]
                    for s in streams:
                        if s < _BS:
                            dst = An[:, s * T + ch * TCH:
                                     s * T + (ch + 1) * TCH]
                        else:
                            b = s - _BS
                            dst = Z2f[:, 4 + b * T + ch * TCH:
                                      4 + b * T + (ch + 1) * TCH]
                        nc.vector.tensor_tensor(
                            dst,
                            h3[:, :, s:s + 1].rearrange("p t o -> p (t o)"),
                            ps3[:, :, s:s + 1].rearrange("p t o -> p (t o)"),
                            op=OP.mult)

                def mkpsb(ch):
                    psb = ps_nrm.tile([128, FC], f32, tag="nps")
                    nc.tensor.matmul(out=psb[:], lhsT=ones_rowb[:],
                                     rhs=rin[:, ch * FC:(ch + 1) * FC],
                                     start=True, stop=True)
                    return psb
                psbs = []
                for ch in range(NFC):
                    psb = mkpsb(ch)
                    psbs.append(psb)
                    pack(range(_BS, S), ch, psb)

                # ---------- gather z2 -> Bn: one-hot shift-accumulate ----
                # Bn[:,t] = sum_d mask_d[t] * Z2f[:, 4+t+d]; the clamped
                # index computation guarantees masked-out terms never pick
                # the padding or a neighboring row's data
                nc.vector.tensor_tensor(Bn[:], Z2f[:, 1:NLOC + 1],
                                        masks[0], op=OP.mult)
                for dd in range(1, 2 * _W + 2):
                    nc.vector.tensor_tensor(
                        tmpb, Z2f[:, 1 + dd:NLOC + 1 + dd],
                        masks[dd], op=OP.mult)
                    nc.vector.tensor_add(Bn[:], Bn[:], tmpb)
                dma(out=ag_in_d.ap(), in_=Bn[:])
                nc.gpsimd.collective_compute(
                    "AllGather", OP.bypass,
                    replica_groups=[list(range(_NCORES))],
                    ins=[ag_in_d.ap().opt()],
                    outs=[ag_out_d.ap().opt()])

                # pack An while the collectives run
                for ch in range(NFC):
                    pack(range(_BS), ch, psbs[ch])

            Ball = bigbuf.tile([128, NGLOB], bf16, tag="ball")
            for jj in range(_NCORES):
                dma(out=Ball[:, jj * NLOC:(jj + 1) * NLOC],
                    in_=ag_out_d.ap()[jj * 128:(jj + 1) * 128, :])

            # ---------- output partials ----------
            outp = consts.tile([128, 4], f32)
            nc.vector.memset(outp[:], 0.0)

            with (
                tc.tile_pool(name="ps_d", bufs=1, space="PSUM") as ps_d,
                tc.tile_pool(name="ps_s", bufs=2, space="PSUM") as ps_s,
                tc.tile_pool(name="ps_c", bufs=1, space="PSUM") as ps_c,
            ):
                # diag: sum_i <An_i, Bn_i>
                dg = bigbuf.tile([128, NLOC], bf16, tag="dg")
                nc.vector.tensor_mul(dg[:], An[:], Bn[:])
                ndc = (NLOC + 511) // 512
                psd = ps_d.tile([1, 512], f32, tag="psd")
                for ch in range(ndc):
                    nc.tensor.matmul(out=psd[:], lhsT=ones_col[:],
                                     rhs=dg[:, ch * 512:(ch + 1) * 512],
                                     start=(ch == 0), stop=(ch == ndc - 1))
                nc.vector.reduce_sum(outp[0:1, 2:3], psd[:],
                                     axis=mybir.AxisListType.X)

                # one-pass row+col logsumexp sweep ([128,1024] exp
                # blocks; row sums on DVE, col sums via ones-matmuls)
                CP = 2 * CC
                NCP = NGLOB // CP
                rows = bigbuf.tile([128, NRC * NCP], f32, tag="rows")
                csum = nrm.tile([1, NGLOB], f32, tag="csum")
                for cb in range(NCP):
                    csp0 = ps_c.tile([1, CC], f32, tag="csp0")
                    csp1 = ps_c.tile([1, CC], f32, tag="csp1")
                    for rc in range(NRC):
                        ps = ps_s.tile([128, CP], f32, tag="sps")
                        nc.tensor.matmul(
                            out=ps[:, 0:CC],
                            lhsT=An[:, rc * 128:(rc + 1) * 128],
                            rhs=Ball[:, cb * CP:cb * CP + CC],
                            start=True, stop=True)
                        nc.tensor.matmul(
                            out=ps[:, CC:CP],
                            lhsT=An[:, rc * 128:(rc + 1) * 128],
                            rhs=Ball[:, cb * CP + CC:(cb + 1) * CP],
                            start=True, stop=True)
                        eb = sgp.tile([128, CP], bf16, tag="eb")
                        nc.scalar.activation(
                            eb[:], ps[:], AF.Exp, scale=1.0 / _TEMP,
                            accum_out=rows[:, rc * NCP + cb:
                                           rc * NCP + cb + 1])
                        nc.tensor.matmul(
                            out=csp0[:], lhsT=ones_col[:], rhs=eb[:, 0:CC],
                            start=(rc == 0), stop=(rc == NRC - 1))
                        nc.tensor.matmul(
                            out=csp1[:], lhsT=ones_col[:], rhs=eb[:, CC:CP],
                            start=(rc == 0), stop=(rc == NRC - 1))
                    nc.vector.tensor_copy(csum[:, cb * CP:cb * CP + CC],
                                          csp0[:])
                    nc.vector.tensor_copy(
                        csum[:, cb * CP + CC:(cb + 1) * CP], csp1[:])

                tot = small.tile([128, NRC], f32, tag="tot")
                for rc in range(NRC):
                    nc.vector.reduce_sum(tot[:, rc:rc + 1],
                                         rows[:, rc * NCP:(rc + 1) * NCP],
                                         axis=mybir.AxisListType.X)
                lse = small.tile([128, NRC], f32, tag="lse")
                nc.scalar.activation(lse[:], tot[:], AF.Ln)
                nc.vector.reduce_sum(outp[:, 0:1], lse[:],
                                     axis=mybir.AxisListType.X)

            dma(out=out_d.ap(), in_=outp[:])
            dma(out=cs_d.ap(), in_=csum[:])
            if dbg:
                anb32 = bigbuf.tile([128, NLOC], f32, tag="anb32")
                for i, src in enumerate((An, Bn)):
                    nc.vector.tensor_copy(anb32[:], src[:])
                    dma(out=anb_d.ap()[:, i * NLOC:(i + 1) * NLOC],
                        in_=anb32[:])
                nc.vector.tensor_copy(anb32[:], Z2f[:, 4:NLOC + 4])
                dma(out=anb_d.ap()[:, 2 * NLOC:3 * NLOC], in_=anb32[:])
                nc.vector.tensor_copy(anb32[:], H1t[:, 0:NLOC])
                dma(out=anb_d.ap()[:, 3 * NLOC:4 * NLOC], in_=anb32[:])

    nc.compile()
    return nc


def _host_prep(x1, x2, u, wih0, whh0, bih0, bhh0, wih1, whh1, bih1, bhh1, T):
    """Build per-core input maps (all host work is pure data layout)."""
    import ml_dtypes
    bf16 = ml_dtypes.bfloat16

    # g-gate (kernel position _GTANH) weights/bias are pre-scaled by 2 so
    # a single sigmoid gives u = sigmoid(2g), tanh(g) = 2u - 1
    def gate_stack_T(w):
        # w: [2, 256, K] -> [K, 512]; cols = dir*256 + gperm_gate*64 + j
        K = w.shape[2]
        out = np.empty((K, 512), np.float32)
        for d in range(2):
            for gi, g in enumerate(_GPERM):
                sc = 2.0 if gi == _GTANH else 1.0
                out[:, d * 256 + gi * 64:d * 256 + (gi + 1) * 64] = \
                    sc * w[d, g * 64:(g + 1) * 64, :].T
        return out.astype(bf16)

    def blockdiag(w):
        # w: [2, 256, H] -> [128, 512]; per new-gate [128,128] block-diag
        out = np.zeros((128, 512), np.float32)
        for gi, g in enumerate(_GPERM):
            sc = 2.0 if gi == _GTANH else 1.0
            out[0:_H, gi * 128:gi * 128 + 64] = \
                sc * w[0, g * 64:(g + 1) * 64, :].T
            out[_H:128, gi * 128 + 64:(gi + 1) * 128] = \
                sc * w[1, g * 64:(g + 1) * 64, :].T
        return out.astype(bf16)

    def biases(bi, bh):
        b = bi + bh  # [2, 256]
        out = np.empty((128, 4), np.float32)
        for gi, g in enumerate(_GPERM):
            sc = 2.0 if gi == _GTANH else 1.0
            out[0:_H, gi] = sc * b[0, g * 64:(g + 1) * 64]
            out[_H:128, gi] = sc * b[1, g * 64:(g + 1) * 64]
        return out

    twt = np.tile(np.arange(T, dtype=np.float32), _BS)[None, :]

    shared = {
        "wi0T": np.ascontiguousarray(gate_stack_T(wih0)),
        "wi1T": np.ascontiguousarray(gate_stack_T(wih1)),
        "wh0bd": blockdiag(whh0),
        "wh1bd": blockdiag(whh1),
        "bias0": biases(bih0, bhh0),
        "bias1": biases(bih1, bhh1),
        "tw2": np.ascontiguousarray(twt),
    }
    in_maps = []
    for k in range(_NCORES):
        rows = [x1[2 * k, :T], x1[2 * k + 1, :T], x2[2 * k, :T],
                x2[2 * k + 1, :T]]
        arr = np.stack(rows, axis=2)            # [T, D, S]
        xc = np.ascontiguousarray(
            arr.transpose(1, 0, 2).reshape(_D, T * _S)).astype(bf16)
        xr = np.ascontiguousarray(
            arr[::-1].transpose(1, 0, 2).reshape(_D, T * _S)).astype(bf16)
        m = dict(shared)
        m["x_cat"] = xc
        m["x_rev"] = xr
        m["u2"] = np.ascontiguousarray(
            u[2 * k:2 * k + 2, :T].reshape(1, -1))
        in_maps.append(m)
    return in_maps


def _run(inputs, T=_T, trace=False, dbg=False):
    from concourse import bass_utils
    key = (T, dbg)
    if key not in _cache:
        _cache[key] = _build(T, dbg)
    nc = _cache[key]
    in_maps = _host_prep(T=T, **inputs)
    res = bass_utils.run_bass_kernel_spmd(
        nc, in_maps, core_ids=list(range(_NCORES)), trace=trace)
    N = _NCORES * _BS * T
    R = sum(float(r["outp"][:, 0].sum()) for r in res.results)
    Draw = sum(float(r["outp"][0, 2]) for r in res.results)
    colsum = np.zeros(N, np.float64)
    for r in res.results:
        colsum += np.asarray(r["colsum"][0], np.float64)
    C = float(np.log(colsum).sum())
    Dg = Draw / _TEMP
    loss = -((Dg - R) / N + (Dg - C) / N)
    return np.float32(loss), res


def kernel(**inputs):
    loss, _ = _run(inputs)
    return np.asarray(loss, dtype=np.float32)

